# revision 21
# baseline (speedup 1.0000x reference)
"""Trainium2 Bass kernel for a BasicTransformerBlock (self-attn + cross-attn + GEGLU FF).

Sharding: sequence-parallel over the 8 cores. Core c handles batch b=c//4,
token chunk (c%4)*512 : (c%4+1)*512, feature-major [D, T] on device.
K/V for the full batch are exchanged with an on-device AllGather per 4-core
group. All GEMM operands are bf16 (PSUM accumulation stays fp32); the
residual stream is fp32. Projection outputs that feed attention use a
head-permuted column order (8 blocks of dims 0-127 per head, then 2 blocks
packing the 32-dim per-head remainders) so every attention matmul is a slice
of a few large SBUF tiles, and V carries a ones column per head so the
softmax denominator rides along in the PV matmul.
"""
import sys

import numpy as np

sys.path.insert(0, "/opt/trn_rl_repo")

import ml_dtypes  # noqa: E402

import concourse.bass as bass  # noqa: E402
import concourse.tile as tile  # noqa: E402
from concourse import bacc, mybir  # noqa: E402

F32 = mybir.dt.float32
F32R = mybir.dt.float32r
BF16 = mybir.dt.bfloat16
NPBF = ml_dtypes.bfloat16
AF = mybir.ActivationFunctionType

B, S, DIM, SCTX, CROSS, INNER = 2, 2048, 1280, 77, 768, 5120
HEADS, DH = 8, 160
NCORES = 8
T = (B * S) // NCORES          # 512 tokens per core
GROUP = NCORES // B            # 4 cores per batch
ND = DIM // 128                # 10
NDC = CROSS // 128             # 6
NKT = S // 128                 # 16
NM1 = (2 * INNER) // 128       # 80
NI = INNER // 128              # 40
LN_EPS = 1e-5
ATT_SCALE = DH ** -0.5
DHP = DH + 1                   # v column group padded with a ones column
VW = HEADS * DHP               # 1288
SCP = 80                       # context tokens padded 77 -> 80
NTT = T // 128                 # 4 token tiles per core
NDP = 11                       # head-packed blocks: 8 main + 3 remainder


def _r(ap):
    return ap if ap.dtype in (F32R, BF16) else ap.bitcast(F32R)


def _bslot(h):
    """(block, partition offset) of head h's 32-dim remainder; offsets are
    limited to {0, 32, 64} by the PE base-partition constraint."""
    return 8 + h // 3, (h % 3) * 32


# --------------------------------------------------------------------------
# device-side building blocks
# --------------------------------------------------------------------------

def _consts(nc, cpool):
    ones = cpool.tile([128, 128], F32, tag="ones")
    nc.any.memset(ones[:], 1.0)
    ones_bf = cpool.tile([128, 1], BF16, tag="ones_bf")
    nc.any.memset(ones_bf[:], 1.0)
    eps_t = cpool.tile([1, 1], F32, tag="eps")
    nc.any.memset(eps_t[:], LN_EPS)
    return ones, ones_bf, eps_t


ADASL = (4 * DIM) // NCORES    # 640 output cols of one ada per core


def _ada_local(nc, tc, emb_ap, w_ap, b_ap, spool, dram_pool):
    """Both adaLN embeddings computed locally on every core (no collective).
    Returns (s2d1, onep1, s2d2, onep2) as [128, 2*ND]/[128, ND] images."""
    emb_sb = spool.tile([128, 2 * ND], F32, tag="emb_sb")
    nc.sync.dma_start(emb_sb[:], emb_ap[:])
    semb = spool.tile([128, 2 * ND], BF16, tag="semb")
    with nc.allow_low_precision(reason="bf16 ada"):
        nc.scalar.activation(semb[:], emb_sb[:], AF.Silu)
    scr = dram_pool.tile([4 * DIM], F32)
    with tc.tile_pool(name="ada_w", bufs=2) as awpool, \
         tc.tile_pool(name="ada_tmp", bufs=2) as atmp, \
         tc.tile_pool(name="ada_b", bufs=1) as abpool, \
         tc.tile_pool(name="ada_ps", bufs=2, space="PSUM") as app:
        b_t = abpool.tile([1, 4 * DIM], F32, tag="ada_bt")
        nc.sync.dma_start(b_t[:],
                          b_ap[:].rearrange("(o n) -> o n", o=1))
        for blk in range(ND):
            wt = awpool.tile([128, ND * 512], BF16, tag="adaw")
            nc.sync.dma_start(wt[:], w_ap[blk])
            ps = app.tile([1, 512], F32, tag="stat")
            co = 0 if blk < ND // 2 else ND
            for d in range(ND):
                nc.tensor.matmul(ps[:], semb[:, co + d:co + d + 1],
                                 wt[:, d * 512:(d + 1) * 512],
                                 start=(d == 0), stop=(d == ND - 1))
            ssb = atmp.tile([1, 512], F32, tag="ada_s")
            nc.vector.tensor_add(ssb[:], ps[:],
                                 b_t[:, blk * 512:(blk + 1) * 512])
            nc.sync.dma_start(scr[blk * 512:(blk + 1) * 512], ssb[:])
    out = []
    for idx in range(2):
        s2d = spool.tile([128, 2 * ND], F32, tag=f"s2d{idx}")
        nc.sync.dma_start(
            s2d[:], scr[idx * 2 * DIM:(idx + 1) * 2 * DIM]
            .rearrange("(j p) -> p j", p=128))
        onep = spool.tile([128, ND], F32, tag=f"onep{idx}")
        nc.vector.tensor_scalar_add(onep[:], s2d[:, 0:ND], 1.0)
        out += [s2d, onep]
    return out


def _layernorm(nc, tc, x_t, n, scale_fn, shift_fn, out_pool, out_tag,
               ones, eps_t, ones_stat=None, sq_dt=F32R):
    """Feature-major LN over len(x_t) tiles [128, n] + per-feature affine.
    Returns bf16 tiles. ones_stat must match the x/sq dtype."""
    nd = len(x_t)
    ones_col = ones_stat if ones_stat is not None \
        else ones[:, 0:1].bitcast(F32R)
    with tc.tile_pool(name="ln_s", bufs=1) as spool, \
         tc.tile_pool(name="ln_tmp", bufs=3) as tmp_pool, \
         tc.tile_pool(name="ln_ps", bufs=2, space="PSUM") as pp_stat, \
         tc.tile_pool(name="ln_bc", bufs=2, space="PSUM") as pp_bc:
        ps_sum = pp_stat.tile([1, n], F32, tag="stat")
        for j in range(nd):
            nc.tensor.matmul(ps_sum[:], ones_col, _r(x_t[j][:]),
                             start=(j == 0), stop=(j == nd - 1))
        ps_sq = pp_stat.tile([1, n], F32, tag="stat")
        for j in range(nd):
            sq = tmp_pool.tile([128, n], sq_dt, tag="ln_sq")
            with nc.allow_low_precision(reason="bf16 sq for LN stats"):
                nc.scalar.activation(sq[:], x_t[j][:], AF.Square)
            nc.tensor.matmul(ps_sq[:], ones_col, sq[:],
                             start=(j == 0), stop=(j == nd - 1))
        mean = spool.tile([1, n], F32R, tag="ln_mean")
        nc.scalar.activation(mean[:], ps_sum[:], AF.Copy,
                             scale=1.0 / (nd * 128))
        msq = spool.tile([1, n], F32, tag="ln_msq")
        nc.scalar.activation(msq[:], ps_sq[:], AF.Copy,
                             scale=1.0 / (nd * 128))
        m2 = spool.tile([1, n], F32, tag="ln_m2")
        nc.vector.tensor_mul(m2[:], mean[:], mean[:])
        var = spool.tile([1, n], F32, tag="ln_var")
        nc.vector.tensor_sub(var[:], msq[:], m2[:])
        std = spool.tile([1, n], F32, tag="ln_std")
        nc.scalar.activation(std[:], var[:], AF.Sqrt, bias=eps_t[:])
        rstd = spool.tile([1, n], F32R, tag="ln_rstd")
        with nc.allow_low_precision(reason="rstd feeds fp32r bcast matmul"):
            nc.vector.reciprocal(rstd[:], std[:])
        ps_mb = pp_bc.tile([128, n], F32, tag="bcast")
        nc.tensor.matmul(ps_mb[:], ones[0:1, :].bitcast(F32R), mean[:],
                         start=True, stop=True)
        ps_rb = pp_bc.tile([128, n], F32, tag="bcast")
        nc.tensor.matmul(ps_rb[:], ones[0:1, :].bitcast(F32R), rstd[:],
                         start=True, stop=True)
        h_t = []
        for j in range(nd):
            xc = tmp_pool.tile([128, n], F32, tag="ln_xc")
            nc.vector.tensor_sub(xc[:], x_t[j][:], ps_mb[:])
            xn = tmp_pool.tile([128, n], F32, tag="ln_xn")
            nc.vector.tensor_mul(xn[:], xc[:], ps_rb[:])
            h = out_pool.tile([128, n], BF16, tag=out_tag)
            with nc.allow_low_precision(reason="bf16 gemm operands"):
                nc.scalar.activation(h[:], xn[:], AF.Identity,
                                     bias=shift_fn(j), scale=scale_fn(j))
            h_t.append(h)
        return h_t


def _proj_mtiles(nc, wimg_pool, pp, img_ap, h_t, out_cb, tag, nm=NDP):
    """m-tile projection: out[m] = sum_d w[d,m]^T h[d]; out_cb(m, ps)."""
    for m in range(nm):
        wt = wimg_pool.tile([128, DIM], BF16, tag=tag)
        nc.sync.dma_start(wt[:], img_ap[m])
        ps = pp.tile([128, T], F32, tag="mm")
        for d in range(ND):
            nc.tensor.matmul(ps[:], wt[:, d * 128:(d + 1) * 128],
                             h_t[d][:], start=(d == 0), stop=(d == ND - 1))
        out_cb(m, ps)


def _out_proj(nc, pp, stage, xpool, wimg_pool, o_pk, wo_img, bias_col,
              x_prev_fn, x_tag):
    """Attn out-projection from packed o tiles + bias + residual."""
    x_new = []
    for m in range(ND):
        wt = wimg_pool.tile([128, NDP * 128], BF16, tag="wimg")
        nc.sync.dma_start(wt[:], wo_img[m])
        ps = pp.tile([128, T], F32, tag="mm")
        for b in range(NDP):
            nc.tensor.matmul(ps[:], wt[:, b * 128:(b + 1) * 128],
                             o_pk[b][:], start=(b == 0),
                             stop=(b == NDP - 1))
        t1 = stage.tile([128, T], F32, tag="t1")
        nc.scalar.activation(t1[:], ps[:], AF.Identity, bias=bias_col(m))
        xn = xpool.tile([128, T], F32R, tag=x_tag)
        with nc.allow_low_precision(reason="residual stream fp32r"):
            nc.vector.tensor_add(xn[:], t1[:], x_prev_fn(m))
        x_new.append(xn)
    return x_new


def _attn_core(nc, tc, pools, kslice_a, kslice_b, vslice_a, vslice_b,
               q_a, q_b, nkt, kpart, e_tag):
    """Shared attention inner loop: per head scores->exp->PV->normalize.
    Returns packed o tiles (8 a-blocks + 2 b-blocks), bf16."""
    (opool, epool, apool, pp, pp_pva, pp_pvb, pp_bc, ones) = pools
    o_pk = [opool.tile([128, T], BF16, tag=f"opk{i}", name=f"opk{i}")
            for i in range(8)]
    o_pkb = [opool.tile([128, T], BF16, tag=f"opkb{i}", name=f"opkb{i}")
             for i in range(3)]
    nc.vector.memset(o_pkb[0][96:128, :], 0.0)
    nc.vector.memset(o_pkb[1][96:128, :], 0.0)
    nc.vector.memset(o_pkb[2][64:128, :], 0.0)
    def normalize(h, ps_a, ps_b):
        rt = apool.tile([33, T], F32R, tag="recip")
        with nc.allow_low_precision(reason="softmax recip"):
            nc.vector.reciprocal(rt[32:33, :], ps_b[32:33, :])
        ps_rb = pp_bc.tile([128, T], F32, tag="bcast")
        nc.tensor.matmul(ps_rb[:], ones[32:33, :].bitcast(F32R),
                         rt[32:33, :], start=True, stop=True)
        rb = apool.tile([128, T], F32, tag="rb")
        nc.scalar.copy(rb[:], ps_rb[:])
        blk, r0 = _bslot(h)
        with nc.allow_low_precision(reason="attn out bf16"):
            nc.vector.tensor_mul(o_pk[h][:], ps_a[:], rb[:])
            nc.vector.tensor_mul(o_pkb[blk - 8][r0:r0 + 32, :],
                                 ps_b[0:32, :], rb[r0:r0 + 32, :])

    prev = None
    for h in range(HEADS):
        e_t = []
        for kt in range(nkt):
            if prev is not None and kt < nkt:
                ph, pe, pa, pb = prev
                nc.tensor.matmul(pa[:], vslice_a(ph, kt), pe[kt][:],
                                 start=(kt == 0), stop=(kt == nkt - 1))
                nc.tensor.matmul(pb[:], vslice_b(ph, kt), pe[kt][:],
                                 start=(kt == 0), stop=(kt == nkt - 1))
            ps = pp.tile([kpart, T], F32, tag="mm")
            nc.tensor.matmul(ps[:], kslice_a(h, kt), q_a(h),
                             start=True, stop=False)
            nc.tensor.matmul(ps[:], kslice_b(h, kt), q_b(h),
                             start=False, stop=True)
            ex = epool.tile([kpart, T], BF16, tag=e_tag)
            with nc.allow_low_precision(reason="bf16 probs"):
                nc.scalar.activation(ex[:], ps[:], AF.Exp, scale=ATT_SCALE)
            e_t.append(ex)
        if prev is not None:
            normalize(prev[0], prev[2], prev[3])
        ps_a = pp_pva.tile([128, T], F32, tag="pva")
        ps_b = pp_pvb.tile([33, T], F32, tag="pvb")
        prev = (h, e_t, ps_a, ps_b)
    ph, pe, pa, pb = prev
    for kt in range(nkt):
        nc.tensor.matmul(pa[:], vslice_a(ph, kt), pe[kt][:],
                         start=(kt == 0), stop=(kt == nkt - 1))
        nc.tensor.matmul(pb[:], vslice_b(ph, kt), pe[kt][:],
                         start=(kt == 0), stop=(kt == nkt - 1))
    normalize(ph, pa, pb)
    return o_pk + o_pkb


# --------------------------------------------------------------------------
# the single-launch program
# --------------------------------------------------------------------------

def _build():
    nc = bacc.Bacc("TRN2", target_bir_lowering=False, debug=False,
                   num_devices=NCORES)
    P = nc.declare_dram_parameter
    t = {}
    t["xT"] = P("xT", [DIM, T], BF16, isOutput=False)
    t["ctxT"] = P("ctxT", [CROSS, SCP], BF16, isOutput=False)
    t["emb_img"] = P("emb_img", [128, 2 * ND], F32, isOutput=False)
    t["ada_w_img"] = P("ada_w_img", [ND, 128, ND * 512], BF16,
                       isOutput=False)
    t["ada_b_img"] = P("ada_b_img", [4 * DIM], F32, isOutput=False)
    t["wq_img"] = P("wq_img", [NDP, 128, DIM], BF16, isOutput=False)
    t["wk_img"] = P("wk_img", [NDP, 128, DIM], BF16, isOutput=False)
    t["wv_img"] = P("wv_img", [ND, 128, VW], BF16, isOutput=False)
    t["wo1_img"] = P("wo1_img", [ND, 128, NDP * 128], BF16, isOutput=False)
    t["wq2_img"] = P("wq2_img", [NDP, 128, DIM], BF16, isOutput=False)
    t["wk2_img"] = P("wk2_img", [NDP, 128, CROSS], BF16, isOutput=False)
    t["wv2_img"] = P("wv2_img", [NDC, 128, VW], BF16, isOutput=False)
    t["wo2_img"] = P("wo2_img", [ND, 128, NDP * 128], BF16, isOutput=False)
    t["biases_img"] = P("biases_img", [128, 4 * ND], F32, isOutput=False)
    t["w1_img"] = P("w1_img", [NM1, 128, DIM], BF16, isOutput=False)
    t["b1_img"] = P("b1_img", [128, NM1], F32, isOutput=False)
    t["w2_img"] = P("w2_img", [ND, 128, INNER], BF16, isOutput=False)
    t["b2_img"] = P("b2_img", [128, ND], F32, isOutput=False)
    t["yT"] = P("yT", [DIM, T], F32, isOutput=True)

    with tile.TileContext(nc) as tc:
        _kernel_body(nc, tc, t)
    nc.compile()
    return nc


def _kernel_body(nc, tc, t):
    import contextlib
    with contextlib.ExitStack() as es:
        e = es.enter_context
        cpool = e(tc.tile_pool(name="const", bufs=1))
        spool = e(tc.tile_pool(name="spool", bufs=1))
        stage = e(tc.tile_pool(name="stage", bufs=2))
        xpool3 = e(tc.tile_pool(name="xp3", bufs=ND))
        dram_pool = e(tc.tile_pool(name="dram", bufs=1, space="DRAM"))

        ones, ones_bf, eps_t = _consts(nc, cpool)
        biases = cpool.tile([128, 4 * ND], F32, tag="biases")
        nc.sync.dma_start(biases[:], t["biases_img"][:])
        b1_t = cpool.tile([128, NM1], F32, tag="b1")
        nc.sync.dma_start(b1_t[:], t["b1_img"][:])
        b2_t = cpool.tile([128, ND], F32, tag="b2")
        nc.sync.dma_start(b2_t[:], t["b2_img"][:])

        x0 = cpool.tile([128, ND * T], BF16, tag="x0")
        nc.sync.dma_start(x0[:].rearrange("p (b t) -> p b t", t=T),
                          t["xT"][:].rearrange("(b p) t -> p b t", p=128))
        x_t = [x0[:, j * T:(j + 1) * T] for j in range(ND)]

        # ---------------- cross-attn context K/V (independent; early) ------
        ctx_sb = cpool.tile([128, NDC * SCP], BF16, tag="ctx")
        nc.sync.dma_start(ctx_sb[:].rearrange("p (d s) -> p d s", s=SCP),
                          t["ctxT"][:].rearrange("(d p) s -> p d s", p=128))
        ctx_t = [ctx_sb[:, d * SCP:(d + 1) * SCP] for d in range(NDC)]
        # ---------------- adaLN embeddings ----------------
        s2d1, onep1, s2d2, onep2 = _ada_local(
            nc, tc, t["emb_img"], t["ada_w_img"], t["ada_b_img"],
            spool, dram_pool)

        KSP = 6                       # K AG split: blocks 0-5 / 6-10
        kstg = [dram_pool.tile([KSP * 128, T], BF16, name="kstg0"),
                dram_pool.tile([(NDP - KSP) * 128, T], BF16, name="kstg1")]
        vstg = [dram_pool.tile([2 * 128, VW], BF16, name="vstg0"),
                dram_pool.tile([2 * 128, VW], BF16, name="vstg1")]
        kgth = [dram_pool.tile([GROUP, KSP * 128, T], BF16, name="kgth0"),
                dram_pool.tile([GROUP, (NDP - KSP) * 128, T], BF16,
                               name="kgth1")]
        vgth = [dram_pool.tile([GROUP, 2 * 128, VW], BF16, name="vgth0"),
                dram_pool.tile([GROUP, 2 * 128, VW], BF16, name="vgth1")]
        groups = [[0, 1, 2, 3], [4, 5, 6, 7]]

        # ---------------- LN1 + q/k/v projections ----------------
        xp2_es = contextlib.ExitStack()
        xpool2 = xp2_es.enter_context(tc.tile_pool(name="xp2", bufs=ND))
        kvq_es = contextlib.ExitStack()
        q_sb = kvq_es.enter_context(
            tc.tile_pool(name="qp", bufs=1)).tile(
            [128, NDP * T], BF16, tag="q")
        with tc.tile_pool(name="hp", bufs=ND) as hpool, \
             tc.tile_pool(name="wimg", bufs=3) as wimg, \
             tc.tile_pool(name="kv1", bufs=1) as kv1pool, \
             tc.tile_pool(name="kv", bufs=2) as kvpool, \
             tc.tile_pool(name="ppa", bufs=3, space="PSUM") as pp:
            h_t = _layernorm(nc, tc, x_t, T,
                             lambda j: onep1[:, j:j + 1],
                             lambda j: s2d1[:, ND + j:ND + j + 1],
                             hpool, "h", ones, eps_t,
                             ones_stat=ones_bf[:, 0:1], sq_dt=BF16)

            k_all = kv1pool.tile([128, NDP * T], BF16, tag="kall")

            def k_out(m, ps):
                with nc.allow_low_precision(reason="bf16 k"):
                    nc.vector.tensor_copy(k_all[:, m * T:(m + 1) * T],
                                          ps[:])

            for m in range(NDP):
                wt = wimg.tile([128, DIM], BF16, tag="wimg")
                nc.sync.dma_start(wt[:], t["wk_img"][m])
                ps = pp.tile([128, T], F32, tag="mm")
                for d in range(ND):
                    nc.tensor.matmul(ps[:], wt[:, d * 128:(d + 1) * 128],
                                     h_t[d][:], start=(d == 0),
                                     stop=(d == ND - 1))
                k_out(m, ps)
                if m == KSP - 1 or m == NDP - 1:
                    half = 0 if m == KSP - 1 else 1
                    c0 = 0 if half == 0 else KSP * T
                    nb = KSP if half == 0 else NDP - KSP
                    nc.sync.dma_start(
                        kstg[half][:].rearrange("(b p) t -> p b t", p=128),
                        k_all[:, c0:c0 + nb * T]
                        .rearrange("p (b t) -> p b t", t=T))
                    nc.gpsimd.collective_compute(
                        "AllGather", mybir.AluOpType.bypass,
                        replica_groups=groups, ins=[kstg[half][:]],
                        outs=[kgth[half][:]])

            wv_sb = kv1pool.tile([128, ND * VW], BF16, tag="wv")
            nc.sync.dma_start(
                wv_sb[:].rearrange("p (d c) -> p d c", c=VW),
                t["wv_img"][:].rearrange("d p c -> p d c"))
            for tt in range(NTT):
                vtile = kvpool.tile([128, VW], BF16, tag="vtile")
                for off, nn in ((0, 512), (512, 512), (1024, VW - 1024)):
                    ps = pp.tile([128, 512], F32, tag="mm")
                    for d in range(ND):
                        nc.tensor.matmul(
                            ps[:, 0:nn], h_t[d][:, tt * 128:(tt + 1) * 128],
                            wv_sb[:, d * VW + off:d * VW + off + nn],
                            start=(d == 0), stop=(d == ND - 1))
                    with nc.allow_low_precision(reason="bf16 v"):
                        nc.scalar.copy(vtile[:, off:off + nn], ps[:, 0:nn])
                for h in range(HEADS):
                    nc.vector.memset(vtile[:, h * DHP + DH:(h + 1) * DHP],
                                     1.0)
                half, r = tt // 2, (tt % 2) * 128
                nc.sync.dma_start(vstg[half][r:r + 128, :], vtile[:])
                if tt % 2 == 1:
                    nc.gpsimd.collective_compute(
                        "AllGather", mybir.AluOpType.bypass,
                        replica_groups=groups, ins=[vstg[half][:]],
                        outs=[vgth[half][:]])

            def q_out(m, ps):
                with nc.allow_low_precision(reason="bf16 q"):
                    nc.vector.tensor_copy(q_sb[:, m * T:(m + 1) * T],
                                          ps[:])

            _proj_mtiles(nc, wimg, pp, t["wq_img"], h_t, q_out, "wimg")

        # ---------------- cross-attn context K/V (fills the AG gap) --
        k2_sb = cpool.tile([128, NDP * SCP], BF16, tag="k2")
        v2pad = cpool.tile([SCP, VW], BF16, tag="v2pad")
        with tc.tile_pool(name="cw", bufs=3) as cwpool, \
             tc.tile_pool(name="cwv1", bufs=1) as cwvpool, \
             tc.tile_pool(name="cps", bufs=2, space="PSUM") as cpp:
            for m in range(NDP):
                wt = cwpool.tile([128, CROSS], BF16, tag="cw")
                nc.sync.dma_start(wt[:], t["wk2_img"][m])
                ps = cpp.tile([128, SCP], F32, tag="cmm")
                for d in range(NDC):
                    nc.tensor.matmul(ps[:], wt[:, d * 128:(d + 1) * 128],
                                     ctx_t[d], start=(d == 0),
                                     stop=(d == NDC - 1))
                with nc.allow_low_precision(reason="bf16 k2"):
                    nc.scalar.copy(k2_sb[:, m * SCP:(m + 1) * SCP], ps[:])
            wv2_sb = cwvpool.tile([128, NDC * VW], BF16, tag="cwv")
            nc.sync.dma_start(
                wv2_sb[:].rearrange("p (d c) -> p d c", c=VW),
                t["wv2_img"][:].rearrange("d p c -> p d c"))
            for off, nn in ((0, 512), (512, 512), (1024, VW - 1024)):
                ps = cpp.tile([SCP, 512], F32, tag="cmm2")
                for d in range(NDC):
                    nc.tensor.matmul(ps[:, 0:nn], ctx_t[d],
                                     wv2_sb[:, d * VW + off:d * VW + off + nn],
                                     start=(d == 0), stop=(d == NDC - 1))
                with nc.allow_low_precision(reason="bf16 v2"):
                    nc.scalar.copy(v2pad[:, off:off + nn], ps[:, 0:nn])
            # ones columns for the denominator (rows 77:80 stay zero: ctx
            # padding is zero so the psum wrote zeros there, and the host
            # zeroed the wv2 ones-columns)
            for h in range(HEADS):
                nc.vector.memset(v2pad[0:SCTX, h * DHP + DH:(h + 1) * DHP],
                                 1.0)

        # ---------------- self-attention ----------------
        with tc.tile_pool(name="att_o", bufs=1) as opool, \
             tc.tile_pool(name="att_e", bufs=NKT + 3) as epool, \
             tc.tile_pool(name="att_s", bufs=1) as apool, \
             tc.tile_pool(name="wimg", bufs=3) as wimg, \
             tc.tile_pool(name="ppa", bufs=3, space="PSUM") as pp, \
             tc.tile_pool(name="ppv", bufs=2, space="PSUM") as pp_pva, \
             tc.tile_pool(name="ppw", bufs=2, space="PSUM") as pp_pvb, \
             tc.tile_pool(name="ppb", bufs=1, space="PSUM") as pp_bc:
            gkv_es = contextlib.ExitStack()
            gkvpool = gkv_es.enter_context(tc.tile_pool(name="gkv", bufs=1))
            kt_sb = [gkvpool.tile([128, NDP * T], BF16, tag=f"ktg{g}",
                                  name=f"ktg{g}") for g in range(GROUP)]
            vt_sb = [gkvpool.tile([128, NTT * VW], BF16, tag=f"vtg{g}",
                                  name=f"vtg{g}") for g in range(GROUP)]
            for g in range(GROUP):
                nc.sync.dma_start(
                    kt_sb[g][:, 0:KSP * T]
                    .rearrange("p (b t) -> p b t", t=T),
                    kgth[0][g].rearrange("(b p) t -> p b t", p=128))
                nc.sync.dma_start(
                    kt_sb[g][:, KSP * T:]
                    .rearrange("p (b t) -> p b t", t=T),
                    kgth[1][g].rearrange("(b p) t -> p b t", p=128))
            for g in range(GROUP):
                nc.sync.dma_start(
                    vt_sb[g][:, 0:2 * VW]
                    .rearrange("p (j c) -> p j c", c=VW),
                    vgth[0][g].rearrange("(j p) c -> p j c", p=128))
                nc.sync.dma_start(
                    vt_sb[g][:, 2 * VW:]
                    .rearrange("p (j c) -> p j c", c=VW),
                    vgth[1][g].rearrange("(j p) c -> p j c", p=128))

            def kslice_a(h, kt):
                g, j = kt // NTT, kt % NTT
                return kt_sb[g][:, h * T + j * 128:h * T + (j + 1) * 128]

            def kslice_b(h, kt):
                g, j = kt // NTT, kt % NTT
                blk, r0 = _bslot(h)
                return kt_sb[g][r0:r0 + 32,
                                blk * T + j * 128:blk * T + (j + 1) * 128]

            def vslice_a(h, kt):
                g, j = kt // NTT, kt % NTT
                return vt_sb[g][:, j * VW + h * DHP:j * VW + h * DHP + 128]

            def vslice_b(h, kt):
                g, j = kt // NTT, kt % NTT
                return vt_sb[g][:, j * VW + h * DHP + 128:
                                j * VW + (h + 1) * DHP]

            def q_a(h):
                return q_sb[:, h * T:(h + 1) * T]

            def q_b(h):
                blk, r0 = _bslot(h)
                return q_sb[r0:r0 + 32, blk * T:(blk + 1) * T]

            pools = (opool, epool, apool, pp, pp_pva, pp_pvb, pp_bc, ones)
            o_pk = _attn_core(nc, tc, pools, kslice_a, kslice_b,
                              vslice_a, vslice_b, q_a, q_b, NKT, 128, "e1")
            gkv_es.close()
            x2_t = _out_proj(nc, pp, stage, xpool2, wimg, o_pk,
                             t["wo1_img"],
                             lambda m: biases[:, m:m + 1],
                             lambda m: x_t[m], "x2")
        kvq_es.close()

        # ---------------- cross-attention ----------------
        q2_es = contextlib.ExitStack()
        q2_sb = q2_es.enter_context(
            tc.tile_pool(name="q2p", bufs=1)).tile(
            [128, NDP * T], BF16, tag="q2")
        with tc.tile_pool(name="hp", bufs=ND) as hpool, \
             tc.tile_pool(name="wimg", bufs=3) as wimg, \
             tc.tile_pool(name="cr_o", bufs=1) as opool, \
             tc.tile_pool(name="cr_e", bufs=4) as epool, \
             tc.tile_pool(name="cr_s", bufs=2) as apool:
            h2_t = _layernorm(nc, tc, x2_t, T,
                              lambda j: onep2[:, j:j + 1],
                              lambda j: s2d2[:, ND + j:ND + j + 1],
                              hpool, "h", ones, eps_t)

            def q2_out(m, ps):
                with nc.allow_low_precision(reason="bf16 q2"):
                    nc.vector.tensor_copy(q2_sb[:, m * T:(m + 1) * T],
                                          ps[:])

            with tc.tile_pool(name="ppq2", bufs=3, space="PSUM") as ppq2:
                _proj_mtiles(nc, wimg, ppq2, t["wq2_img"], h2_t, q2_out,
                             "wimg")

            def k2slice_a(h, kt):
                return k2_sb[:, h * SCP:(h + 1) * SCP]

            def k2slice_b(h, kt):
                blk, r0 = _bslot(h)
                return k2_sb[r0:r0 + 32, blk * SCP:(blk + 1) * SCP]

            def v2slice_a(h, kt):
                return v2pad[:, h * DHP:h * DHP + 128]

            def v2slice_b(h, kt):
                return v2pad[:, h * DHP + 128:(h + 1) * DHP]

            def q2_a(h):
                return q2_sb[:, h * T:(h + 1) * T]

            def q2_b(h):
                blk, r0 = _bslot(h)
                return q2_sb[r0:r0 + 32, blk * T:(blk + 1) * T]

            with tc.tile_pool(name="ppa", bufs=2, space="PSUM") as pp, \
                 tc.tile_pool(name="ppv", bufs=2, space="PSUM") as pp_pva, \
                 tc.tile_pool(name="ppw", bufs=2, space="PSUM") as pp_pvb, \
                 tc.tile_pool(name="ppb", bufs=1, space="PSUM") as pp_bc:
                pools = (opool, epool, apool, pp, pp_pva, pp_pvb, pp_bc,
                         ones)
                o2_pk = _attn_core(nc, tc, pools, k2slice_a, k2slice_b,
                                   v2slice_a, v2slice_b, q2_a, q2_b, 1, SCP,
                                   "e2")
                x3_t = _out_proj(nc, pp, stage, xpool3, wimg, o2_pk,
                                 t["wo2_img"],
                                 lambda m: biases[:, ND + m:ND + m + 1],
                                 lambda m: x2_t[m][:], "x3")
        q2_es.close()
        xp2_es.close()

        # ---------------- GEGLU feed-forward ----------------
        with tc.tile_pool(name="hp", bufs=ND) as hpool, \
             tc.tile_pool(name="wimg", bufs=4) as wimg, \
             tc.tile_pool(name="ff_hg", bufs=NI) as hgpool, \
             tc.tile_pool(name="ff_u", bufs=3) as upool, \
             tc.tile_pool(name="ff_w2", bufs=2) as w2pool, \
             tc.tile_pool(name="ppa", bufs=4, space="PSUM") as pp:
            h3_t = _layernorm(nc, tc, x3_t, T,
                              lambda j: biases[:, 2 * ND + j:2 * ND + j + 1],
                              lambda j: biases[:, 3 * ND + j:3 * ND + j + 1],
                              hpool, "h", ones, eps_t)
            hg_t = []
            for i in range(NI):
                wt = wimg.tile([128, 2 * DIM], BF16, tag="w1pair")
                nc.sync.dma_start(
                    wt[:].rearrange("p (i c) -> p i c", c=DIM),
                    t["w1_img"][2 * i:2 * i + 2].rearrange("i p c -> p i c"))
                ps_u = pp.tile([128, T], F32, tag="mm")
                for d in range(ND):
                    nc.tensor.matmul(ps_u[:], wt[:, d * 128:(d + 1) * 128],
                                     h3_t[d][:], start=(d == 0),
                                     stop=(d == ND - 1))
                ps_g = pp.tile([128, T], F32, tag="mm")
                for d in range(ND):
                    nc.tensor.matmul(ps_g[:],
                                     wt[:, DIM + d * 128:DIM + (d + 1) * 128],
                                     h3_t[d][:], start=(d == 0),
                                     stop=(d == ND - 1))
                u = upool.tile([128, T], F32, tag="u")
                nc.scalar.activation(u[:], ps_u[:], AF.Identity,
                                     bias=b1_t[:, 2 * i:2 * i + 1])
                g = upool.tile([128, T], F32, tag="g")
                nc.scalar.activation(g[:], ps_g[:], AF.Gelu,
                                     bias=b1_t[:, 2 * i + 1:2 * i + 2])
                hg = hgpool.tile([128, T], BF16, tag="hg")
                with nc.allow_low_precision(reason="bf16 geglu"):
                    nc.vector.tensor_mul(hg[:], u[:], g[:])
                hg_t.append(hg)
            for m in range(ND):
                wt = w2pool.tile([128, INNER], BF16, tag="w2")
                nc.sync.dma_start(wt[:], t["w2_img"][m])
                ps = pp.tile([128, T], F32, tag="mm")
                for d in range(NI):
                    nc.tensor.matmul(ps[:], wt[:, d * 128:(d + 1) * 128],
                                     hg_t[d][:], start=(d == 0),
                                     stop=(d == NI - 1))
                t1 = stage.tile([128, T], F32, tag="t1")
                nc.scalar.activation(t1[:], ps[:], AF.Identity,
                                     bias=b2_t[:, m:m + 1])
                y = stage.tile([128, T], F32, tag="y")
                nc.vector.tensor_add(y[:], t1[:], x3_t[m][:])
                nc.sync.dma_start(t["yT"][m * 128:(m + 1) * 128, :], y[:])


# --------------------------------------------------------------------------
# host side: weight images
# --------------------------------------------------------------------------

def _head_perm_pad():
    """Padded column map [NDP*128]: 8 blocks of head dims 0-127, then 3
    blocks packing per-head 32-dim remainders at offsets 0/32/64; -1 = zero
    padding."""
    idx = np.full(NDP * 128, -1, np.int64)
    for h in range(HEADS):
        idx[h * 128:(h + 1) * 128] = np.arange(h * DH, h * DH + 128)
    for h in range(HEADS):
        blk, off = _bslot(h)
        idx[blk * 128 + off:blk * 128 + off + 32] = np.arange(
            h * DH + 128, h * DH + DH)
    return idx


def _pad_cols(w, idx):
    out = np.zeros((w.shape[0], len(idx)), w.dtype)
    sel = idx >= 0
    out[:, sel] = w[:, idx[sel]]
    return out


def _pad_rows(w, idx):
    out = np.zeros((len(idx), w.shape[1]), w.dtype)
    sel = idx >= 0
    out[sel, :] = w[idx[sel], :]
    return out


def _img_kxm(w, mcols=128):
    """[K, M] weight -> [M//mcols, 128, (K//128)*mcols] m-tile images."""
    K, M = w.shape
    nd, nm = K // 128, M // mcols
    return np.ascontiguousarray(
        w.reshape(nd, 128, nm, mcols).transpose(2, 1, 0, 3)
        .reshape(nm, 128, nd * mcols))


def _v_pad(w):
    """[K, DIM] -> [K, VW] with a zero column after each head's 160 dims."""
    K = w.shape[0]
    out = np.zeros((K, VW), w.dtype)
    for h in range(HEADS):
        out[:, h * DHP:h * DHP + DH] = w[:, h * DH:(h + 1) * DH]
    return out


def _col_img(v):
    """[N] -> [128, N//128] image: img[p, j] = v[j*128 + p]."""
    return np.ascontiguousarray(v.reshape(-1, 128).T)


_STATE = {}

_STATIC_NAMES = (
    "ada_w_img", "ada_b_img", "wq_img", "wk_img", "wv_img", "wo1_img",
    "wq2_img", "wk2_img", "wv2_img", "wo2_img", "biases_img",
    "w1_img", "b1_img", "w2_img", "b2_img",
)


def _prepare(inputs):
    key = tuple(np.asarray(inputs[k]).ctypes.data for k in
                ("a1_wq", "ff_w1", "ff_w2", "a2_wk", "a1_wo"))
    if _STATE.get("key") == key:
        return _STATE["prep"]
    f = np.float32
    idx = _head_perm_pad()
    g = {}

    def bimg(w):
        return _img_kxm(np.asarray(w, f)).astype(NPBF)

    g["wq_img"] = bimg(_pad_cols(np.asarray(inputs["a1_wq"], f), idx))
    g["wk_img"] = bimg(_pad_cols(np.asarray(inputs["a1_wk"], f), idx))
    g["wv_img"] = np.ascontiguousarray(
        _v_pad(np.asarray(inputs["a1_wv"], f)).reshape(ND, 128, VW)
    ).astype(NPBF)
    g["wo1_img"] = bimg(_pad_rows(np.asarray(inputs["a1_wo"], f), idx))
    g["wq2_img"] = bimg(_pad_cols(np.asarray(inputs["a2_wq"], f), idx))
    g["wk2_img"] = bimg(_pad_cols(np.asarray(inputs["a2_wk"], f), idx))
    g["wv2_img"] = np.ascontiguousarray(
        _v_pad(np.asarray(inputs["a2_wv"], f)).reshape(NDC, 128, VW)
    ).astype(NPBF)
    g["wo2_img"] = bimg(_pad_rows(np.asarray(inputs["a2_wo"], f), idx))

    a1w = np.asarray(inputs["ada1_w"], f)
    a1b = np.asarray(inputs["ada1_b"], f)
    a2w = np.asarray(inputs["ada2_w"], f)
    a2b = np.asarray(inputs["ada2_b"], f)
    g["ada_w_img"] = _img_kxm(np.hstack([a1w, a2w]),
                              mcols=512).astype(NPBF)
    g["ada_b_img"] = np.ascontiguousarray(np.concatenate([a1b, a2b]))

    g["biases_img"] = np.ascontiguousarray(np.concatenate([
        _col_img(np.asarray(inputs["a1_bo"], f)),
        _col_img(np.asarray(inputs["a2_bo"], f)),
        _col_img(np.asarray(inputs["norm3_g"], f)),
        _col_img(np.asarray(inputs["norm3_b"], f))], axis=1))

    w1 = np.asarray(inputs["ff_w1"], f)
    w1i = np.stack([w1[:, :INNER].reshape(DIM, NI, 128),
                    w1[:, INNER:].reshape(DIM, NI, 128)],
                   axis=2).reshape(DIM, 2 * INNER)
    g["w1_img"] = bimg(w1i)
    b1 = np.asarray(inputs["ff_b1"], f)
    b1i = np.stack([b1[:INNER].reshape(NI, 128),
                    b1[INNER:].reshape(NI, 128)], axis=1).reshape(-1)
    g["b1_img"] = _col_img(b1i)
    g["w2_img"] = bimg(np.asarray(inputs["ff_w2"], f))
    g["b2_img"] = _col_img(np.asarray(inputs["ff_b2"], f))
    _STATE["key"] = key
    _STATE["prep"] = g
    _STATE.pop("static_dev", None)
    return g


# --------------------------------------------------------------------------
# SPMD runner with device-resident static inputs
# --------------------------------------------------------------------------

class _SpmdRunner:
    """Caches the jitted callable and keeps device-resident global arrays
    for static inputs."""

    def __init__(self, nc, n_cores):
        import functools

        import jax
        import jax.numpy as jnp
        from jax.experimental.shard_map import shard_map
        from jax.sharding import Mesh, NamedSharding, PartitionSpec

        from concourse import bass2jax

        bass2jax.install_neuronx_cc_hook()
        self.jax = jax
        self.nc = nc
        self.n_cores = n_cores
        partition_name = (nc.partition_id_tensor.name
                          if nc.partition_id_tensor else None)
        in_names, out_names, out_avals, zero_shapes = [], [], [], []
        for alloc in nc.m.functions[0].allocations:
            if not isinstance(alloc, mybir.MemoryLocationSet):
                continue
            name = alloc.memorylocations[0].name
            if alloc.kind == "ExternalInput":
                if name != partition_name:
                    in_names.append(name)
            elif alloc.kind == "ExternalOutput":
                shape = tuple(alloc.tensor_shape)
                dtype = mybir.dt.np(alloc.dtype)
                out_names.append(name)
                out_avals.append(jax.core.ShapedArray(shape, dtype))
                zero_shapes.append((shape, dtype))
        self.n_params = len(in_names)
        self.in_names = list(in_names)
        self.out_names = list(out_names)
        self.out_avals = out_avals
        all_in_names = list(in_names) + list(out_names)
        if partition_name is not None:
            all_in_names.append(partition_name)
        donate = tuple(range(self.n_params,
                             self.n_params + len(out_names)))

        def _bdy(*args):
            operands = list(args)
            if partition_name is not None:
                operands.append(bass2jax.partition_id_tensor())
            outs = bass2jax._bass_exec_p.bind(
                *operands,
                out_avals=tuple(out_avals),
                in_names=tuple(all_in_names),
                out_names=tuple(out_names),
                lowering_input_output_aliases=(),
                sim_require_finite=True,
                sim_require_nnan=True,
                nc=nc,
            )
            return tuple(outs)

        devices = jax.devices()[:n_cores]
        self.mesh = Mesh(np.asarray(devices), ("core",))
        self.sharding = NamedSharding(self.mesh, PartitionSpec("core"))
        n_z = len(zero_shapes)
        self.sharded = jax.jit(
            shard_map(_bdy, mesh=self.mesh,
                      in_specs=(PartitionSpec("core"),) * (self.n_params
                                                           + n_z),
                      out_specs=(PartitionSpec("core"),) * len(out_names),
                      check_rep=False),
            donate_argnums=donate, keep_unused=True)
        self._zero_fns = []
        for shape, dtype in zero_shapes:
            gshape = (n_cores * shape[0],) + tuple(shape[1:])
            self._zero_fns.append(jax.jit(
                functools.partial(jnp.zeros, gshape, dtype),
                out_shardings=self.sharding))
        self._static_cache = {}

    def put_static(self, name, per_core_arrays):
        gl = np.concatenate(per_core_arrays, axis=0)
        self._static_cache[name] = self.jax.device_put(gl, self.sharding)

    def __call__(self, in_maps):
        args = []
        for name in self.in_names:
            if name in self._static_cache:
                args.append(self._static_cache[name])
            else:
                gl = np.concatenate(
                    [np.asarray(m[name]) for m in in_maps], axis=0)
                args.append(self.jax.device_put(gl, self.sharding))
        zeros = [zf() for zf in self._zero_fns]
        out_arrs = self.sharded(*args, *zeros)
        res = []
        for c in range(self.n_cores):
            res.append({
                name: np.asarray(out_arrs[i]).reshape(
                    self.n_cores, *self.out_avals[i].shape)[c]
                for i, name in enumerate(self.out_names)})
        return res


def kernel(**inputs):
    if "nc" not in _STATE:
        _STATE["nc"] = _build()
    g = _prepare(inputs)
    if "runner" not in _STATE:
        _STATE["runner"] = _SpmdRunner(_STATE["nc"], NCORES)
    runner = _STATE["runner"]
    if "static_dev" not in _STATE:
        for name in _STATIC_NAMES:
            arrs = g[name]
            if not isinstance(arrs, list):
                arrs = [arrs] * NCORES
            runner.put_static(name, arrs)
        _STATE["static_dev"] = True
    f = np.float32
    x = np.asarray(inputs["hidden_states"], f)
    ctx = np.asarray(inputs["context"], f)
    tstep = int(np.asarray(inputs["timestep"]))
    emb_img = np.ascontiguousarray(np.concatenate(
        [_col_img(np.asarray(inputs["ada1_emb"], f)[tstep]),
         _col_img(np.asarray(inputs["ada2_emb"], f)[tstep])], axis=1))

    xT = [np.ascontiguousarray(x[b].T) for b in range(B)]  # [DIM, S]
    xT_c = [np.ascontiguousarray(xT[c // GROUP][:, (c % GROUP) * T:
                                                (c % GROUP + 1) * T]
                                 ).astype(NPBF)
            for c in range(NCORES)]
    ctxT = []
    for b in range(B):
        cp = np.zeros((CROSS, SCP), f)
        cp[:, :SCTX] = ctx[b].T
        ctxT.append(cp.astype(NPBF))

    in_maps = [{
        "xT": xT_c[c], "ctxT": ctxT[c // GROUP], "emb_img": emb_img,
    } for c in range(NCORES)]
    res = runner(in_maps)

    y = np.empty((B, S, DIM), f)
    for c in range(NCORES):
        b, i = divmod(c, GROUP)
        y[b, i * T:(i + 1) * T, :] = res[c]["yT"].T
    return y


# revision 22
# speedup vs baseline: 1.0060x; 1.0060x over previous
"""Trainium2 Bass kernel for a BasicTransformerBlock (self-attn + cross-attn + GEGLU FF).

Sharding: sequence-parallel over the 8 cores. Core c handles batch b=c//4,
token chunk (c%4)*512 : (c%4+1)*512, feature-major [D, T] on device.
K/V for the full batch are exchanged with an on-device AllGather per 4-core
group. All GEMM operands are bf16 (PSUM accumulation stays fp32); the
residual stream is fp32. Projection outputs that feed attention use a
head-permuted column order (8 blocks of dims 0-127 per head, then 2 blocks
packing the 32-dim per-head remainders) so every attention matmul is a slice
of a few large SBUF tiles, and V carries a ones column per head so the
softmax denominator rides along in the PV matmul.
"""
import sys

import numpy as np

sys.path.insert(0, "/opt/trn_rl_repo")

import ml_dtypes  # noqa: E402

import concourse.bass as bass  # noqa: E402
import concourse.tile as tile  # noqa: E402
from concourse import bacc, mybir  # noqa: E402

F32 = mybir.dt.float32
F32R = mybir.dt.float32r
BF16 = mybir.dt.bfloat16
NPBF = ml_dtypes.bfloat16
AF = mybir.ActivationFunctionType

B, S, DIM, SCTX, CROSS, INNER = 2, 2048, 1280, 77, 768, 5120
HEADS, DH = 8, 160
NCORES = 8
T = (B * S) // NCORES          # 512 tokens per core
GROUP = NCORES // B            # 4 cores per batch
ND = DIM // 128                # 10
NDC = CROSS // 128             # 6
NKT = S // 128                 # 16
NM1 = (2 * INNER) // 128       # 80
NI = INNER // 128              # 40
LN_EPS = 1e-5
ATT_SCALE = DH ** -0.5
DHP = DH + 1                   # v column group padded with a ones column
VW = HEADS * DHP               # 1288
SCP = 80                       # context tokens padded 77 -> 80
NTT = T // 128                 # 4 token tiles per core
NDP = 11                       # head-packed blocks: 8 main + 3 remainder


def _r(ap):
    return ap if ap.dtype in (F32R, BF16) else ap.bitcast(F32R)


def _bslot(h):
    """(block, partition offset) of head h's 32-dim remainder; offsets are
    limited to {0, 32, 64} by the PE base-partition constraint."""
    return 8 + h // 3, (h % 3) * 32


# --------------------------------------------------------------------------
# device-side building blocks
# --------------------------------------------------------------------------

def _consts(nc, cpool):
    ones = cpool.tile([128, 128], F32, tag="ones")
    nc.any.memset(ones[:], 1.0)
    ones_bf = cpool.tile([128, 1], BF16, tag="ones_bf")
    nc.any.memset(ones_bf[:], 1.0)
    eps_t = cpool.tile([1, 1], F32, tag="eps")
    nc.any.memset(eps_t[:], LN_EPS)
    return ones, ones_bf, eps_t


ADASL = (4 * DIM) // NCORES    # 640 output cols of one ada per core


def _ada_local(nc, tc, emb_ap, w_ap, b_ap, spool, dram_pool):
    """Both adaLN embeddings computed locally on every core (no collective).
    Returns (s2d1, onep1, s2d2, onep2) as [128, 2*ND]/[128, ND] images."""
    emb_sb = spool.tile([128, 2 * ND], F32, tag="emb_sb")
    nc.sync.dma_start(emb_sb[:], emb_ap[:])
    semb = spool.tile([128, 2 * ND], BF16, tag="semb")
    with nc.allow_low_precision(reason="bf16 ada"):
        nc.scalar.activation(semb[:], emb_sb[:], AF.Silu)
    scr = dram_pool.tile([4 * DIM], F32)
    with tc.tile_pool(name="ada_w", bufs=2) as awpool, \
         tc.tile_pool(name="ada_tmp", bufs=2) as atmp, \
         tc.tile_pool(name="ada_b", bufs=1) as abpool, \
         tc.tile_pool(name="ada_ps", bufs=2, space="PSUM") as app:
        b_t = abpool.tile([1, 4 * DIM], F32, tag="ada_bt")
        nc.sync.dma_start(b_t[:],
                          b_ap[:].rearrange("(o n) -> o n", o=1))
        for blk in range(ND):
            wt = awpool.tile([128, ND * 512], BF16, tag="adaw")
            nc.sync.dma_start(wt[:], w_ap[blk])
            ps = app.tile([1, 512], F32, tag="stat")
            co = 0 if blk < ND // 2 else ND
            for d in range(ND):
                nc.tensor.matmul(ps[:], semb[:, co + d:co + d + 1],
                                 wt[:, d * 512:(d + 1) * 512],
                                 start=(d == 0), stop=(d == ND - 1))
            ssb = atmp.tile([1, 512], F32, tag="ada_s")
            nc.vector.tensor_add(ssb[:], ps[:],
                                 b_t[:, blk * 512:(blk + 1) * 512])
            nc.sync.dma_start(scr[blk * 512:(blk + 1) * 512], ssb[:])
    out = []
    for idx in range(2):
        s2d = spool.tile([128, 2 * ND], F32, tag=f"s2d{idx}")
        nc.sync.dma_start(
            s2d[:], scr[idx * 2 * DIM:(idx + 1) * 2 * DIM]
            .rearrange("(j p) -> p j", p=128))
        onep = spool.tile([128, ND], F32, tag=f"onep{idx}")
        nc.vector.tensor_scalar_add(onep[:], s2d[:, 0:ND], 1.0)
        out += [s2d, onep]
    return out


def _layernorm(nc, tc, x_t, n, scale_fn, shift_fn, out_pool, out_tag,
               ones, eps_t, ones_stat=None, sq_dt=F32R):
    """Feature-major LN over len(x_t) tiles [128, n] + per-feature affine.
    Returns bf16 tiles. ones_stat must match the x/sq dtype."""
    nd = len(x_t)
    ones_col = ones_stat if ones_stat is not None \
        else ones[:, 0:1].bitcast(F32R)
    with tc.tile_pool(name="ln_s", bufs=1) as spool, \
         tc.tile_pool(name="ln_tmp", bufs=3) as tmp_pool, \
         tc.tile_pool(name="ln_ps", bufs=2, space="PSUM") as pp_stat, \
         tc.tile_pool(name="ln_bc", bufs=2, space="PSUM") as pp_bc:
        ps_sum = pp_stat.tile([1, n], F32, tag="stat")
        for j in range(nd):
            nc.tensor.matmul(ps_sum[:], ones_col, _r(x_t[j][:]),
                             start=(j == 0), stop=(j == nd - 1))
        ps_sq = pp_stat.tile([1, n], F32, tag="stat")
        for j in range(nd):
            sq = tmp_pool.tile([128, n], sq_dt, tag="ln_sq")
            with nc.allow_low_precision(reason="bf16 sq for LN stats"):
                nc.scalar.activation(sq[:], x_t[j][:], AF.Square)
            nc.tensor.matmul(ps_sq[:], ones_col, sq[:],
                             start=(j == 0), stop=(j == nd - 1))
        mean = spool.tile([1, n], F32R, tag="ln_mean")
        nc.scalar.activation(mean[:], ps_sum[:], AF.Copy,
                             scale=1.0 / (nd * 128))
        msq = spool.tile([1, n], F32, tag="ln_msq")
        nc.scalar.activation(msq[:], ps_sq[:], AF.Copy,
                             scale=1.0 / (nd * 128))
        m2 = spool.tile([1, n], F32, tag="ln_m2")
        nc.vector.tensor_mul(m2[:], mean[:], mean[:])
        var = spool.tile([1, n], F32, tag="ln_var")
        nc.vector.tensor_sub(var[:], msq[:], m2[:])
        std = spool.tile([1, n], F32, tag="ln_std")
        nc.scalar.activation(std[:], var[:], AF.Sqrt, bias=eps_t[:])
        rstd = spool.tile([1, n], F32R, tag="ln_rstd")
        with nc.allow_low_precision(reason="rstd feeds fp32r bcast matmul"):
            nc.vector.reciprocal(rstd[:], std[:])
        ps_mb = pp_bc.tile([128, n], F32, tag="bcast")
        nc.tensor.matmul(ps_mb[:], ones[0:1, :].bitcast(F32R), mean[:],
                         start=True, stop=True)
        ps_rb = pp_bc.tile([128, n], F32, tag="bcast")
        nc.tensor.matmul(ps_rb[:], ones[0:1, :].bitcast(F32R), rstd[:],
                         start=True, stop=True)
        h_t = []
        for j in range(nd):
            xc = tmp_pool.tile([128, n], F32, tag="ln_xc")
            nc.vector.tensor_sub(xc[:], x_t[j][:], ps_mb[:])
            xn = tmp_pool.tile([128, n], F32, tag="ln_xn")
            nc.vector.tensor_mul(xn[:], xc[:], ps_rb[:])
            h = out_pool.tile([128, n], BF16, tag=out_tag)
            with nc.allow_low_precision(reason="bf16 gemm operands"):
                nc.scalar.activation(h[:], xn[:], AF.Identity,
                                     bias=shift_fn(j), scale=scale_fn(j))
            h_t.append(h)
        return h_t


def _proj_mtiles(nc, wimg_pool, pp, img_ap, h_t, out_cb, tag, nm=NDP):
    """m-tile projection: out[m] = sum_d w[d,m]^T h[d]; out_cb(m, ps)."""
    for m in range(nm):
        wt = wimg_pool.tile([128, DIM], BF16, tag=tag)
        nc.sync.dma_start(wt[:], img_ap[m])
        ps = pp.tile([128, T], F32, tag="mm")
        for d in range(ND):
            nc.tensor.matmul(ps[:], wt[:, d * 128:(d + 1) * 128],
                             h_t[d][:], start=(d == 0), stop=(d == ND - 1))
        out_cb(m, ps)


def _out_proj(nc, pp, stage, xpool, wimg_pool, o_pk, wo_img, bias_col,
              x_prev_fn, x_tag):
    """Attn out-projection from packed o tiles + bias + residual."""
    x_new = []
    for m in range(ND):
        wt = wimg_pool.tile([128, NDP * 128], BF16, tag="wimg")
        nc.sync.dma_start(wt[:], wo_img[m])
        ps = pp.tile([128, T], F32, tag="mm")
        for b in range(NDP):
            nc.tensor.matmul(ps[:], wt[:, b * 128:(b + 1) * 128],
                             o_pk[b][:], start=(b == 0),
                             stop=(b == NDP - 1))
        t1 = stage.tile([128, T], F32, tag="t1")
        nc.scalar.activation(t1[:], ps[:], AF.Identity, bias=bias_col(m))
        xn = xpool.tile([128, T], F32R, tag=x_tag)
        with nc.allow_low_precision(reason="residual stream fp32r"):
            nc.vector.tensor_add(xn[:], t1[:], x_prev_fn(m))
        x_new.append(xn)
    return x_new


def _attn_core(nc, tc, pools, kslice_a, kslice_b, vslice_a, vslice_b,
               q_a, q_b, nkt, kpart, e_tag):
    """Shared attention inner loop: per head scores->exp->PV->normalize.
    Returns packed o tiles (8 a-blocks + 2 b-blocks), bf16."""
    (opool, epool, apool, pp, pp_pva, pp_pvb, pp_bc, ones) = pools
    o_pk = [opool.tile([128, T], BF16, tag=f"opk{i}", name=f"opk{i}")
            for i in range(8)]
    o_pkb = [opool.tile([128, T], BF16, tag=f"opkb{i}", name=f"opkb{i}")
             for i in range(3)]
    nc.vector.memset(o_pkb[0][96:128, :], 0.0)
    nc.vector.memset(o_pkb[1][96:128, :], 0.0)
    nc.vector.memset(o_pkb[2][64:128, :], 0.0)
    def normalize(h, ps_a, ps_b):
        rt = apool.tile([33, T], F32R, tag="recip")
        with nc.allow_low_precision(reason="softmax recip"):
            nc.vector.reciprocal(rt[32:33, :], ps_b[32:33, :])
        ps_rb = pp_bc.tile([128, T], F32, tag="bcast")
        nc.tensor.matmul(ps_rb[:], ones[32:33, :].bitcast(F32R),
                         rt[32:33, :], start=True, stop=True)
        rb = apool.tile([128, T], F32, tag="rb")
        nc.scalar.copy(rb[:], ps_rb[:])
        blk, r0 = _bslot(h)
        with nc.allow_low_precision(reason="attn out bf16"):
            nc.vector.tensor_mul(o_pk[h][:], ps_a[:], rb[:])
            nc.vector.tensor_mul(o_pkb[blk - 8][r0:r0 + 32, :],
                                 ps_b[0:32, :], rb[r0:r0 + 32, :])

    LAG = 4
    for h in range(HEADS):
        e_t = []
        ps_a = pp_pva.tile([128, T], F32, tag="pva")
        ps_b = pp_pvb.tile([33, T], F32, tag="pvb")

        def emit_pv(kt, ps_a=ps_a, ps_b=ps_b, e_t=e_t, h=h):
            nc.tensor.matmul(ps_a[:], vslice_a(h, kt), e_t[kt][:],
                             start=(kt == 0), stop=(kt == nkt - 1))
            nc.tensor.matmul(ps_b[:], vslice_b(h, kt), e_t[kt][:],
                             start=(kt == 0), stop=(kt == nkt - 1))

        for kt in range(nkt):
            ps = pp.tile([kpart, T], F32, tag="mm")
            nc.tensor.matmul(ps[:], kslice_a(h, kt), q_a(h),
                             start=True, stop=False)
            nc.tensor.matmul(ps[:], kslice_b(h, kt), q_b(h),
                             start=False, stop=True)
            ex = epool.tile([kpart, T], BF16, tag=e_tag)
            with nc.allow_low_precision(reason="bf16 probs"):
                nc.scalar.activation(ex[:], ps[:], AF.Exp, scale=ATT_SCALE)
            e_t.append(ex)
            if kt >= LAG:
                emit_pv(kt - LAG)
        for kt in range(max(0, nkt - LAG), nkt):
            emit_pv(kt)
        normalize(h, ps_a, ps_b)
    return o_pk + o_pkb


# --------------------------------------------------------------------------
# the single-launch program
# --------------------------------------------------------------------------

def _build():
    nc = bacc.Bacc("TRN2", target_bir_lowering=False, debug=False,
                   num_devices=NCORES)
    P = nc.declare_dram_parameter
    t = {}
    t["xT"] = P("xT", [DIM, T], BF16, isOutput=False)
    t["ctxT"] = P("ctxT", [CROSS, SCP], BF16, isOutput=False)
    t["emb_img"] = P("emb_img", [128, 2 * ND], F32, isOutput=False)
    t["ada_w_img"] = P("ada_w_img", [ND, 128, ND * 512], BF16,
                       isOutput=False)
    t["ada_b_img"] = P("ada_b_img", [4 * DIM], F32, isOutput=False)
    t["wq_img"] = P("wq_img", [NDP, 128, DIM], BF16, isOutput=False)
    t["wk_img"] = P("wk_img", [NDP, 128, DIM], BF16, isOutput=False)
    t["wv_img"] = P("wv_img", [ND, 128, VW], BF16, isOutput=False)
    t["wo1_img"] = P("wo1_img", [ND, 128, NDP * 128], BF16, isOutput=False)
    t["wq2_img"] = P("wq2_img", [NDP, 128, DIM], BF16, isOutput=False)
    t["wk2_img"] = P("wk2_img", [NDP, 128, CROSS], BF16, isOutput=False)
    t["wv2_img"] = P("wv2_img", [NDC, 128, VW], BF16, isOutput=False)
    t["wo2_img"] = P("wo2_img", [ND, 128, NDP * 128], BF16, isOutput=False)
    t["biases_img"] = P("biases_img", [128, 4 * ND], F32, isOutput=False)
    t["w1_img"] = P("w1_img", [NM1, 128, DIM], BF16, isOutput=False)
    t["b1_img"] = P("b1_img", [128, NM1], F32, isOutput=False)
    t["w2_img"] = P("w2_img", [ND, 128, INNER], BF16, isOutput=False)
    t["b2_img"] = P("b2_img", [128, ND], F32, isOutput=False)
    t["yT"] = P("yT", [DIM, T], F32, isOutput=True)

    with tile.TileContext(nc) as tc:
        _kernel_body(nc, tc, t)
    nc.compile()
    return nc


def _kernel_body(nc, tc, t):
    import contextlib
    with contextlib.ExitStack() as es:
        e = es.enter_context
        cpool = e(tc.tile_pool(name="const", bufs=1))
        spool = e(tc.tile_pool(name="spool", bufs=1))
        stage = e(tc.tile_pool(name="stage", bufs=2))
        xpool3 = e(tc.tile_pool(name="xp3", bufs=ND))
        dram_pool = e(tc.tile_pool(name="dram", bufs=1, space="DRAM"))

        ones, ones_bf, eps_t = _consts(nc, cpool)
        biases = cpool.tile([128, 4 * ND], F32, tag="biases")
        nc.sync.dma_start(biases[:], t["biases_img"][:])
        b1_t = cpool.tile([128, NM1], F32, tag="b1")
        nc.sync.dma_start(b1_t[:], t["b1_img"][:])
        b2_t = cpool.tile([128, ND], F32, tag="b2")
        nc.sync.dma_start(b2_t[:], t["b2_img"][:])

        x0 = cpool.tile([128, ND * T], BF16, tag="x0")
        nc.sync.dma_start(x0[:].rearrange("p (b t) -> p b t", t=T),
                          t["xT"][:].rearrange("(b p) t -> p b t", p=128))
        x_t = [x0[:, j * T:(j + 1) * T] for j in range(ND)]

        # ---------------- cross-attn context K/V (independent; early) ------
        ctx_sb = cpool.tile([128, NDC * SCP], BF16, tag="ctx")
        nc.sync.dma_start(ctx_sb[:].rearrange("p (d s) -> p d s", s=SCP),
                          t["ctxT"][:].rearrange("(d p) s -> p d s", p=128))
        ctx_t = [ctx_sb[:, d * SCP:(d + 1) * SCP] for d in range(NDC)]
        # ---------------- adaLN embeddings ----------------
        s2d1, onep1, s2d2, onep2 = _ada_local(
            nc, tc, t["emb_img"], t["ada_w_img"], t["ada_b_img"],
            spool, dram_pool)

        KSP = 6                       # K AG split: blocks 0-5 / 6-10
        kstg = [dram_pool.tile([KSP * 128, T], BF16, name="kstg0"),
                dram_pool.tile([(NDP - KSP) * 128, T], BF16, name="kstg1")]
        vstg = [dram_pool.tile([2 * 128, VW], BF16, name="vstg0"),
                dram_pool.tile([2 * 128, VW], BF16, name="vstg1")]
        kgth = [dram_pool.tile([GROUP, KSP * 128, T], BF16, name="kgth0"),
                dram_pool.tile([GROUP, (NDP - KSP) * 128, T], BF16,
                               name="kgth1")]
        vgth = [dram_pool.tile([GROUP, 2 * 128, VW], BF16, name="vgth0"),
                dram_pool.tile([GROUP, 2 * 128, VW], BF16, name="vgth1")]
        groups = [[0, 1, 2, 3], [4, 5, 6, 7]]

        # ---------------- LN1 + q/k/v projections ----------------
        xp2_es = contextlib.ExitStack()
        xpool2 = xp2_es.enter_context(tc.tile_pool(name="xp2", bufs=ND))
        kvq_es = contextlib.ExitStack()
        q_sb = kvq_es.enter_context(
            tc.tile_pool(name="qp", bufs=1)).tile(
            [128, NDP * T], BF16, tag="q")
        with tc.tile_pool(name="hp", bufs=ND) as hpool, \
             tc.tile_pool(name="wimg", bufs=3) as wimg, \
             tc.tile_pool(name="kv1", bufs=1) as kv1pool, \
             tc.tile_pool(name="kv", bufs=2) as kvpool, \
             tc.tile_pool(name="ppa", bufs=3, space="PSUM") as pp:
            h_t = _layernorm(nc, tc, x_t, T,
                             lambda j: onep1[:, j:j + 1],
                             lambda j: s2d1[:, ND + j:ND + j + 1],
                             hpool, "h", ones, eps_t,
                             ones_stat=ones_bf[:, 0:1], sq_dt=BF16)

            k_all = kv1pool.tile([128, NDP * T], BF16, tag="kall")

            def k_out(m, ps):
                with nc.allow_low_precision(reason="bf16 k"):
                    nc.vector.tensor_copy(k_all[:, m * T:(m + 1) * T],
                                          ps[:])

            for m in range(NDP):
                wt = wimg.tile([128, DIM], BF16, tag="wimg")
                nc.sync.dma_start(wt[:], t["wk_img"][m])
                ps = pp.tile([128, T], F32, tag="mm")
                for d in range(ND):
                    nc.tensor.matmul(ps[:], wt[:, d * 128:(d + 1) * 128],
                                     h_t[d][:], start=(d == 0),
                                     stop=(d == ND - 1))
                k_out(m, ps)
                if m == KSP - 1 or m == NDP - 1:
                    half = 0 if m == KSP - 1 else 1
                    c0 = 0 if half == 0 else KSP * T
                    nb = KSP if half == 0 else NDP - KSP
                    nc.sync.dma_start(
                        kstg[half][:].rearrange("(b p) t -> p b t", p=128),
                        k_all[:, c0:c0 + nb * T]
                        .rearrange("p (b t) -> p b t", t=T))
                    nc.gpsimd.collective_compute(
                        "AllGather", mybir.AluOpType.bypass,
                        replica_groups=groups, ins=[kstg[half][:]],
                        outs=[kgth[half][:]])

            def q_out(m, ps):
                with nc.allow_low_precision(reason="bf16 q"):
                    nc.vector.tensor_copy(q_sb[:, m * T:(m + 1) * T],
                                          ps[:])

            _proj_mtiles(nc, wimg, pp, t["wq_img"], h_t, q_out, "wimg")
            wv_sb = kv1pool.tile([128, ND * VW], BF16, tag="wv")
            nc.sync.dma_start(
                wv_sb[:].rearrange("p (d c) -> p d c", c=VW),
                t["wv_img"][:].rearrange("d p c -> p d c"))
            for tt in range(NTT):
                vtile = kvpool.tile([128, VW], BF16, tag="vtile")
                for off, nn in ((0, 512), (512, 512), (1024, VW - 1024)):
                    ps = pp.tile([128, 512], F32, tag="mm")
                    for d in range(ND):
                        nc.tensor.matmul(
                            ps[:, 0:nn], h_t[d][:, tt * 128:(tt + 1) * 128],
                            wv_sb[:, d * VW + off:d * VW + off + nn],
                            start=(d == 0), stop=(d == ND - 1))
                    with nc.allow_low_precision(reason="bf16 v"):
                        nc.scalar.copy(vtile[:, off:off + nn], ps[:, 0:nn])
                for h in range(HEADS):
                    nc.vector.memset(vtile[:, h * DHP + DH:(h + 1) * DHP],
                                     1.0)
                half, r = tt // 2, (tt % 2) * 128
                nc.sync.dma_start(vstg[half][r:r + 128, :], vtile[:])
                if tt % 2 == 1:
                    nc.gpsimd.collective_compute(
                        "AllGather", mybir.AluOpType.bypass,
                        replica_groups=groups, ins=[vstg[half][:]],
                        outs=[vgth[half][:]])


        # ---------------- cross-attn context K/V (fills the AG gap) --
        k2_sb = cpool.tile([128, NDP * SCP], BF16, tag="k2")
        v2pad = cpool.tile([SCP, VW], BF16, tag="v2pad")
        with tc.tile_pool(name="cw", bufs=3) as cwpool, \
             tc.tile_pool(name="cwv1", bufs=1) as cwvpool, \
             tc.tile_pool(name="cps", bufs=2, space="PSUM") as cpp:
            for m in range(NDP):
                wt = cwpool.tile([128, CROSS], BF16, tag="cw")
                nc.sync.dma_start(wt[:], t["wk2_img"][m])
                ps = cpp.tile([128, SCP], F32, tag="cmm")
                for d in range(NDC):
                    nc.tensor.matmul(ps[:], wt[:, d * 128:(d + 1) * 128],
                                     ctx_t[d], start=(d == 0),
                                     stop=(d == NDC - 1))
                with nc.allow_low_precision(reason="bf16 k2"):
                    nc.scalar.copy(k2_sb[:, m * SCP:(m + 1) * SCP], ps[:])
            wv2_sb = cwvpool.tile([128, NDC * VW], BF16, tag="cwv")
            nc.sync.dma_start(
                wv2_sb[:].rearrange("p (d c) -> p d c", c=VW),
                t["wv2_img"][:].rearrange("d p c -> p d c"))
            for off, nn in ((0, 512), (512, 512), (1024, VW - 1024)):
                ps = cpp.tile([SCP, 512], F32, tag="cmm2")
                for d in range(NDC):
                    nc.tensor.matmul(ps[:, 0:nn], ctx_t[d],
                                     wv2_sb[:, d * VW + off:d * VW + off + nn],
                                     start=(d == 0), stop=(d == NDC - 1))
                with nc.allow_low_precision(reason="bf16 v2"):
                    nc.scalar.copy(v2pad[:, off:off + nn], ps[:, 0:nn])
            # ones columns for the denominator (rows 77:80 stay zero: ctx
            # padding is zero so the psum wrote zeros there, and the host
            # zeroed the wv2 ones-columns)
            for h in range(HEADS):
                nc.vector.memset(v2pad[0:SCTX, h * DHP + DH:(h + 1) * DHP],
                                 1.0)

        # ---------------- self-attention ----------------
        with tc.tile_pool(name="att_o", bufs=1) as opool, \
             tc.tile_pool(name="att_e", bufs=10) as epool, \
             tc.tile_pool(name="att_s", bufs=2) as apool, \
             tc.tile_pool(name="wimg", bufs=3) as wimg, \
             tc.tile_pool(name="ppa", bufs=3, space="PSUM") as pp, \
             tc.tile_pool(name="ppv", bufs=2, space="PSUM") as pp_pva, \
             tc.tile_pool(name="ppw", bufs=2, space="PSUM") as pp_pvb, \
             tc.tile_pool(name="ppb", bufs=1, space="PSUM") as pp_bc:
            gkv_es = contextlib.ExitStack()
            gkvpool = gkv_es.enter_context(tc.tile_pool(name="gkv", bufs=1))
            kt_sb = [gkvpool.tile([128, NDP * T], BF16, tag=f"ktg{g}",
                                  name=f"ktg{g}") for g in range(GROUP)]
            vt_sb = [gkvpool.tile([128, NTT * VW], BF16, tag=f"vtg{g}",
                                  name=f"vtg{g}") for g in range(GROUP)]
            for g in range(GROUP):
                nc.sync.dma_start(
                    kt_sb[g][:, 0:KSP * T]
                    .rearrange("p (b t) -> p b t", t=T),
                    kgth[0][g].rearrange("(b p) t -> p b t", p=128))
                nc.sync.dma_start(
                    kt_sb[g][:, KSP * T:]
                    .rearrange("p (b t) -> p b t", t=T),
                    kgth[1][g].rearrange("(b p) t -> p b t", p=128))
            for g in range(GROUP):
                nc.sync.dma_start(
                    vt_sb[g][:, 0:2 * VW]
                    .rearrange("p (j c) -> p j c", c=VW),
                    vgth[0][g].rearrange("(j p) c -> p j c", p=128))
                nc.sync.dma_start(
                    vt_sb[g][:, 2 * VW:]
                    .rearrange("p (j c) -> p j c", c=VW),
                    vgth[1][g].rearrange("(j p) c -> p j c", p=128))

            def kslice_a(h, kt):
                g, j = kt // NTT, kt % NTT
                return kt_sb[g][:, h * T + j * 128:h * T + (j + 1) * 128]

            def kslice_b(h, kt):
                g, j = kt // NTT, kt % NTT
                blk, r0 = _bslot(h)
                return kt_sb[g][r0:r0 + 32,
                                blk * T + j * 128:blk * T + (j + 1) * 128]

            def vslice_a(h, kt):
                g, j = kt // NTT, kt % NTT
                return vt_sb[g][:, j * VW + h * DHP:j * VW + h * DHP + 128]

            def vslice_b(h, kt):
                g, j = kt // NTT, kt % NTT
                return vt_sb[g][:, j * VW + h * DHP + 128:
                                j * VW + (h + 1) * DHP]

            def q_a(h):
                return q_sb[:, h * T:(h + 1) * T]

            def q_b(h):
                blk, r0 = _bslot(h)
                return q_sb[r0:r0 + 32, blk * T:(blk + 1) * T]

            pools = (opool, epool, apool, pp, pp_pva, pp_pvb, pp_bc, ones)
            o_pk = _attn_core(nc, tc, pools, kslice_a, kslice_b,
                              vslice_a, vslice_b, q_a, q_b, NKT, 128, "e1")
            gkv_es.close()
            x2_t = _out_proj(nc, pp, stage, xpool2, wimg, o_pk,
                             t["wo1_img"],
                             lambda m: biases[:, m:m + 1],
                             lambda m: x_t[m], "x2")
        kvq_es.close()

        # ---------------- cross-attention ----------------
        q2_es = contextlib.ExitStack()
        q2_sb = q2_es.enter_context(
            tc.tile_pool(name="q2p", bufs=1)).tile(
            [128, NDP * T], BF16, tag="q2")
        with tc.tile_pool(name="hp", bufs=ND) as hpool, \
             tc.tile_pool(name="wimg", bufs=3) as wimg, \
             tc.tile_pool(name="cr_o", bufs=1) as opool, \
             tc.tile_pool(name="cr_e", bufs=4) as epool, \
             tc.tile_pool(name="cr_s", bufs=2) as apool:
            h2_t = _layernorm(nc, tc, x2_t, T,
                              lambda j: onep2[:, j:j + 1],
                              lambda j: s2d2[:, ND + j:ND + j + 1],
                              hpool, "h", ones, eps_t)

            def q2_out(m, ps):
                with nc.allow_low_precision(reason="bf16 q2"):
                    nc.vector.tensor_copy(q2_sb[:, m * T:(m + 1) * T],
                                          ps[:])

            with tc.tile_pool(name="ppq2", bufs=3, space="PSUM") as ppq2:
                _proj_mtiles(nc, wimg, ppq2, t["wq2_img"], h2_t, q2_out,
                             "wimg")

            def k2slice_a(h, kt):
                return k2_sb[:, h * SCP:(h + 1) * SCP]

            def k2slice_b(h, kt):
                blk, r0 = _bslot(h)
                return k2_sb[r0:r0 + 32, blk * SCP:(blk + 1) * SCP]

            def v2slice_a(h, kt):
                return v2pad[:, h * DHP:h * DHP + 128]

            def v2slice_b(h, kt):
                return v2pad[:, h * DHP + 128:(h + 1) * DHP]

            def q2_a(h):
                return q2_sb[:, h * T:(h + 1) * T]

            def q2_b(h):
                blk, r0 = _bslot(h)
                return q2_sb[r0:r0 + 32, blk * T:(blk + 1) * T]

            with tc.tile_pool(name="ppa", bufs=2, space="PSUM") as pp, \
                 tc.tile_pool(name="ppv", bufs=2, space="PSUM") as pp_pva, \
                 tc.tile_pool(name="ppw", bufs=2, space="PSUM") as pp_pvb, \
                 tc.tile_pool(name="ppb", bufs=1, space="PSUM") as pp_bc:
                pools = (opool, epool, apool, pp, pp_pva, pp_pvb, pp_bc,
                         ones)
                o2_pk = _attn_core(nc, tc, pools, k2slice_a, k2slice_b,
                                   v2slice_a, v2slice_b, q2_a, q2_b, 1, SCP,
                                   "e2")
                x3_t = _out_proj(nc, pp, stage, xpool3, wimg, o2_pk,
                                 t["wo2_img"],
                                 lambda m: biases[:, ND + m:ND + m + 1],
                                 lambda m: x2_t[m][:], "x3")
        q2_es.close()
        xp2_es.close()

        # ---------------- GEGLU feed-forward ----------------
        with tc.tile_pool(name="hp", bufs=ND) as hpool, \
             tc.tile_pool(name="wimg", bufs=4) as wimg, \
             tc.tile_pool(name="ff_hg", bufs=NI) as hgpool, \
             tc.tile_pool(name="ff_u", bufs=3) as upool, \
             tc.tile_pool(name="ff_w2", bufs=2) as w2pool, \
             tc.tile_pool(name="ppa", bufs=4, space="PSUM") as pp:
            h3_t = _layernorm(nc, tc, x3_t, T,
                              lambda j: biases[:, 2 * ND + j:2 * ND + j + 1],
                              lambda j: biases[:, 3 * ND + j:3 * ND + j + 1],
                              hpool, "h", ones, eps_t)
            hg_t = []
            for i in range(NI):
                wt = wimg.tile([128, 2 * DIM], BF16, tag="w1pair")
                nc.sync.dma_start(
                    wt[:].rearrange("p (i c) -> p i c", c=DIM),
                    t["w1_img"][2 * i:2 * i + 2].rearrange("i p c -> p i c"))
                ps_u = pp.tile([128, T], F32, tag="mm")
                for d in range(ND):
                    nc.tensor.matmul(ps_u[:], wt[:, d * 128:(d + 1) * 128],
                                     h3_t[d][:], start=(d == 0),
                                     stop=(d == ND - 1))
                ps_g = pp.tile([128, T], F32, tag="mm")
                for d in range(ND):
                    nc.tensor.matmul(ps_g[:],
                                     wt[:, DIM + d * 128:DIM + (d + 1) * 128],
                                     h3_t[d][:], start=(d == 0),
                                     stop=(d == ND - 1))
                u = upool.tile([128, T], F32, tag="u")
                nc.scalar.activation(u[:], ps_u[:], AF.Identity,
                                     bias=b1_t[:, 2 * i:2 * i + 1])
                g = upool.tile([128, T], F32, tag="g")
                nc.scalar.activation(g[:], ps_g[:], AF.Gelu,
                                     bias=b1_t[:, 2 * i + 1:2 * i + 2])
                hg = hgpool.tile([128, T], BF16, tag="hg")
                with nc.allow_low_precision(reason="bf16 geglu"):
                    nc.vector.tensor_mul(hg[:], u[:], g[:])
                hg_t.append(hg)
            for m in range(ND):
                wt = w2pool.tile([128, INNER], BF16, tag="w2")
                nc.sync.dma_start(wt[:], t["w2_img"][m])
                ps = pp.tile([128, T], F32, tag="mm")
                for d in range(NI):
                    nc.tensor.matmul(ps[:], wt[:, d * 128:(d + 1) * 128],
                                     hg_t[d][:], start=(d == 0),
                                     stop=(d == NI - 1))
                t1 = stage.tile([128, T], F32, tag="t1")
                nc.scalar.activation(t1[:], ps[:], AF.Identity,
                                     bias=b2_t[:, m:m + 1])
                y = stage.tile([128, T], F32, tag="y")
                nc.vector.tensor_add(y[:], t1[:], x3_t[m][:])
                nc.sync.dma_start(t["yT"][m * 128:(m + 1) * 128, :], y[:])


# --------------------------------------------------------------------------
# host side: weight images
# --------------------------------------------------------------------------

def _head_perm_pad():
    """Padded column map [NDP*128]: 8 blocks of head dims 0-127, then 3
    blocks packing per-head 32-dim remainders at offsets 0/32/64; -1 = zero
    padding."""
    idx = np.full(NDP * 128, -1, np.int64)
    for h in range(HEADS):
        idx[h * 128:(h + 1) * 128] = np.arange(h * DH, h * DH + 128)
    for h in range(HEADS):
        blk, off = _bslot(h)
        idx[blk * 128 + off:blk * 128 + off + 32] = np.arange(
            h * DH + 128, h * DH + DH)
    return idx


def _pad_cols(w, idx):
    out = np.zeros((w.shape[0], len(idx)), w.dtype)
    sel = idx >= 0
    out[:, sel] = w[:, idx[sel]]
    return out


def _pad_rows(w, idx):
    out = np.zeros((len(idx), w.shape[1]), w.dtype)
    sel = idx >= 0
    out[sel, :] = w[idx[sel], :]
    return out


def _img_kxm(w, mcols=128):
    """[K, M] weight -> [M//mcols, 128, (K//128)*mcols] m-tile images."""
    K, M = w.shape
    nd, nm = K // 128, M // mcols
    return np.ascontiguousarray(
        w.reshape(nd, 128, nm, mcols).transpose(2, 1, 0, 3)
        .reshape(nm, 128, nd * mcols))


def _v_pad(w):
    """[K, DIM] -> [K, VW] with a zero column after each head's 160 dims."""
    K = w.shape[0]
    out = np.zeros((K, VW), w.dtype)
    for h in range(HEADS):
        out[:, h * DHP:h * DHP + DH] = w[:, h * DH:(h + 1) * DH]
    return out


def _col_img(v):
    """[N] -> [128, N//128] image: img[p, j] = v[j*128 + p]."""
    return np.ascontiguousarray(v.reshape(-1, 128).T)


_STATE = {}

_STATIC_NAMES = (
    "ada_w_img", "ada_b_img", "wq_img", "wk_img", "wv_img", "wo1_img",
    "wq2_img", "wk2_img", "wv2_img", "wo2_img", "biases_img",
    "w1_img", "b1_img", "w2_img", "b2_img",
)


def _prepare(inputs):
    key = tuple(np.asarray(inputs[k]).ctypes.data for k in
                ("a1_wq", "ff_w1", "ff_w2", "a2_wk", "a1_wo"))
    if _STATE.get("key") == key:
        return _STATE["prep"]
    f = np.float32
    idx = _head_perm_pad()
    g = {}

    def bimg(w):
        return _img_kxm(np.asarray(w, f)).astype(NPBF)

    g["wq_img"] = bimg(_pad_cols(np.asarray(inputs["a1_wq"], f), idx))
    g["wk_img"] = bimg(_pad_cols(np.asarray(inputs["a1_wk"], f), idx))
    g["wv_img"] = np.ascontiguousarray(
        _v_pad(np.asarray(inputs["a1_wv"], f)).reshape(ND, 128, VW)
    ).astype(NPBF)
    g["wo1_img"] = bimg(_pad_rows(np.asarray(inputs["a1_wo"], f), idx))
    g["wq2_img"] = bimg(_pad_cols(np.asarray(inputs["a2_wq"], f), idx))
    g["wk2_img"] = bimg(_pad_cols(np.asarray(inputs["a2_wk"], f), idx))
    g["wv2_img"] = np.ascontiguousarray(
        _v_pad(np.asarray(inputs["a2_wv"], f)).reshape(NDC, 128, VW)
    ).astype(NPBF)
    g["wo2_img"] = bimg(_pad_rows(np.asarray(inputs["a2_wo"], f), idx))

    a1w = np.asarray(inputs["ada1_w"], f)
    a1b = np.asarray(inputs["ada1_b"], f)
    a2w = np.asarray(inputs["ada2_w"], f)
    a2b = np.asarray(inputs["ada2_b"], f)
    g["ada_w_img"] = _img_kxm(np.hstack([a1w, a2w]),
                              mcols=512).astype(NPBF)
    g["ada_b_img"] = np.ascontiguousarray(np.concatenate([a1b, a2b]))

    g["biases_img"] = np.ascontiguousarray(np.concatenate([
        _col_img(np.asarray(inputs["a1_bo"], f)),
        _col_img(np.asarray(inputs["a2_bo"], f)),
        _col_img(np.asarray(inputs["norm3_g"], f)),
        _col_img(np.asarray(inputs["norm3_b"], f))], axis=1))

    w1 = np.asarray(inputs["ff_w1"], f)
    w1i = np.stack([w1[:, :INNER].reshape(DIM, NI, 128),
                    w1[:, INNER:].reshape(DIM, NI, 128)],
                   axis=2).reshape(DIM, 2 * INNER)
    g["w1_img"] = bimg(w1i)
    b1 = np.asarray(inputs["ff_b1"], f)
    b1i = np.stack([b1[:INNER].reshape(NI, 128),
                    b1[INNER:].reshape(NI, 128)], axis=1).reshape(-1)
    g["b1_img"] = _col_img(b1i)
    g["w2_img"] = bimg(np.asarray(inputs["ff_w2"], f))
    g["b2_img"] = _col_img(np.asarray(inputs["ff_b2"], f))
    _STATE["key"] = key
    _STATE["prep"] = g
    _STATE.pop("static_dev", None)
    return g


# --------------------------------------------------------------------------
# SPMD runner with device-resident static inputs
# --------------------------------------------------------------------------

class _SpmdRunner:
    """Caches the jitted callable and keeps device-resident global arrays
    for static inputs."""

    def __init__(self, nc, n_cores):
        import functools

        import jax
        import jax.numpy as jnp
        from jax.experimental.shard_map import shard_map
        from jax.sharding import Mesh, NamedSharding, PartitionSpec

        from concourse import bass2jax

        bass2jax.install_neuronx_cc_hook()
        self.jax = jax
        self.nc = nc
        self.n_cores = n_cores
        partition_name = (nc.partition_id_tensor.name
                          if nc.partition_id_tensor else None)
        in_names, out_names, out_avals, zero_shapes = [], [], [], []
        for alloc in nc.m.functions[0].allocations:
            if not isinstance(alloc, mybir.MemoryLocationSet):
                continue
            name = alloc.memorylocations[0].name
            if alloc.kind == "ExternalInput":
                if name != partition_name:
                    in_names.append(name)
            elif alloc.kind == "ExternalOutput":
                shape = tuple(alloc.tensor_shape)
                dtype = mybir.dt.np(alloc.dtype)
                out_names.append(name)
                out_avals.append(jax.core.ShapedArray(shape, dtype))
                zero_shapes.append((shape, dtype))
        self.n_params = len(in_names)
        self.in_names = list(in_names)
        self.out_names = list(out_names)
        self.out_avals = out_avals
        all_in_names = list(in_names) + list(out_names)
        if partition_name is not None:
            all_in_names.append(partition_name)
        donate = tuple(range(self.n_params,
                             self.n_params + len(out_names)))

        def _bdy(*args):
            operands = list(args)
            if partition_name is not None:
                operands.append(bass2jax.partition_id_tensor())
            outs = bass2jax._bass_exec_p.bind(
                *operands,
                out_avals=tuple(out_avals),
                in_names=tuple(all_in_names),
                out_names=tuple(out_names),
                lowering_input_output_aliases=(),
                sim_require_finite=True,
                sim_require_nnan=True,
                nc=nc,
            )
            return tuple(outs)

        devices = jax.devices()[:n_cores]
        self.mesh = Mesh(np.asarray(devices), ("core",))
        self.sharding = NamedSharding(self.mesh, PartitionSpec("core"))
        n_z = len(zero_shapes)
        self.sharded = jax.jit(
            shard_map(_bdy, mesh=self.mesh,
                      in_specs=(PartitionSpec("core"),) * (self.n_params
                                                           + n_z),
                      out_specs=(PartitionSpec("core"),) * len(out_names),
                      check_rep=False),
            donate_argnums=donate, keep_unused=True)
        self._zero_fns = []
        for shape, dtype in zero_shapes:
            gshape = (n_cores * shape[0],) + tuple(shape[1:])
            self._zero_fns.append(jax.jit(
                functools.partial(jnp.zeros, gshape, dtype),
                out_shardings=self.sharding))
        self._static_cache = {}

    def put_static(self, name, per_core_arrays):
        gl = np.concatenate(per_core_arrays, axis=0)
        self._static_cache[name] = self.jax.device_put(gl, self.sharding)

    def __call__(self, in_maps):
        args = []
        for name in self.in_names:
            if name in self._static_cache:
                args.append(self._static_cache[name])
            else:
                gl = np.concatenate(
                    [np.asarray(m[name]) for m in in_maps], axis=0)
                args.append(self.jax.device_put(gl, self.sharding))
        zeros = [zf() for zf in self._zero_fns]
        out_arrs = self.sharded(*args, *zeros)
        res = []
        for c in range(self.n_cores):
            res.append({
                name: np.asarray(out_arrs[i]).reshape(
                    self.n_cores, *self.out_avals[i].shape)[c]
                for i, name in enumerate(self.out_names)})
        return res


def kernel(**inputs):
    if "nc" not in _STATE:
        _STATE["nc"] = _build()
    g = _prepare(inputs)
    if "runner" not in _STATE:
        _STATE["runner"] = _SpmdRunner(_STATE["nc"], NCORES)
    runner = _STATE["runner"]
    if "static_dev" not in _STATE:
        for name in _STATIC_NAMES:
            arrs = g[name]
            if not isinstance(arrs, list):
                arrs = [arrs] * NCORES
            runner.put_static(name, arrs)
        _STATE["static_dev"] = True
    f = np.float32
    x = np.asarray(inputs["hidden_states"], f)
    ctx = np.asarray(inputs["context"], f)
    tstep = int(np.asarray(inputs["timestep"]))
    emb_img = np.ascontiguousarray(np.concatenate(
        [_col_img(np.asarray(inputs["ada1_emb"], f)[tstep]),
         _col_img(np.asarray(inputs["ada2_emb"], f)[tstep])], axis=1))

    xT = [np.ascontiguousarray(x[b].T) for b in range(B)]  # [DIM, S]
    xT_c = [np.ascontiguousarray(xT[c // GROUP][:, (c % GROUP) * T:
                                                (c % GROUP + 1) * T]
                                 ).astype(NPBF)
            for c in range(NCORES)]
    ctxT = []
    for b in range(B):
        cp = np.zeros((CROSS, SCP), f)
        cp[:, :SCTX] = ctx[b].T
        ctxT.append(cp.astype(NPBF))

    in_maps = [{
        "xT": xT_c[c], "ctxT": ctxT[c // GROUP], "emb_img": emb_img,
    } for c in range(NCORES)]
    res = runner(in_maps)

    y = np.empty((B, S, DIM), f)
    for c in range(NCORES):
        b, i = divmod(c, GROUP)
        y[b, i * T:(i + 1) * T, :] = res[c]["yT"].T
    return y


# revision 24
# speedup vs baseline: 1.0927x; 1.0861x over previous
"""Trainium2 Bass kernel for a BasicTransformerBlock (self-attn + cross-attn + GEGLU FF).

Sharding: sequence-parallel over the 8 cores. Core c handles batch b=c//4,
token chunk (c%4)*512 : (c%4+1)*512, feature-major [D, T] on device.
K/V for the full batch are exchanged with an on-device AllGather per 4-core
group. All GEMM operands are bf16 (PSUM accumulation stays fp32); the
residual stream is fp32. Projection outputs that feed attention use a
head-permuted column order (8 blocks of dims 0-127 per head, then 2 blocks
packing the 32-dim per-head remainders) so every attention matmul is a slice
of a few large SBUF tiles, and V carries a ones column per head so the
softmax denominator rides along in the PV matmul.
"""
import sys

import numpy as np

sys.path.insert(0, "/opt/trn_rl_repo")

import ml_dtypes  # noqa: E402

import concourse.bass as bass  # noqa: E402
import concourse.tile as tile  # noqa: E402
from concourse import bacc, mybir  # noqa: E402

F32 = mybir.dt.float32
F32R = mybir.dt.float32r
BF16 = mybir.dt.bfloat16
NPBF = ml_dtypes.bfloat16
AF = mybir.ActivationFunctionType

B, S, DIM, SCTX, CROSS, INNER = 2, 2048, 1280, 77, 768, 5120
HEADS, DH = 8, 160
NCORES = 8
T = (B * S) // NCORES          # 512 tokens per core
GROUP = NCORES // B            # 4 cores per batch
ND = DIM // 128                # 10
NDC = CROSS // 128             # 6
NKT = S // 128                 # 16
NM1 = (2 * INNER) // 128       # 80
NI = INNER // 128              # 40
LN_EPS = 1e-5
ATT_SCALE = DH ** -0.5
DHP = DH + 1                   # v column group padded with a ones column
VW = HEADS * DHP               # 1288
SCP = 80                       # context tokens padded 77 -> 80
NTT = T // 128                 # 4 token tiles per core
NDP = 11                       # head-packed blocks: 8 main + 3 remainder


def _r(ap):
    return ap if ap.dtype in (F32R, BF16) else ap.bitcast(F32R)


def _bslot(h):
    """(block, partition offset) of head h's 32-dim remainder; offsets are
    limited to {0, 32, 64} by the PE base-partition constraint."""
    return 8 + h // 3, (h % 3) * 32


# --------------------------------------------------------------------------
# device-side building blocks
# --------------------------------------------------------------------------

def _consts(nc, cpool):
    ones = cpool.tile([128, 128], F32, tag="ones")
    nc.any.memset(ones[:], 1.0)
    ones_bf = cpool.tile([128, 1], BF16, tag="ones_bf")
    nc.any.memset(ones_bf[:], 1.0)
    eps_t = cpool.tile([1, 1], F32, tag="eps")
    nc.any.memset(eps_t[:], LN_EPS)
    return ones, ones_bf, eps_t


ADASL = (4 * DIM) // NCORES    # 640 output cols of one ada per core


def _ada_local(nc, tc, emb_ap, w_ap, b_ap, spool, dram_pool):
    """Both adaLN embeddings computed locally on every core (no collective).
    Returns (s2d1, onep1, s2d2, onep2) as [128, 2*ND]/[128, ND] images."""
    emb_sb = spool.tile([128, 2 * ND], F32, tag="emb_sb")
    nc.sync.dma_start(emb_sb[:], emb_ap[:])
    semb = spool.tile([128, 2 * ND], BF16, tag="semb")
    with nc.allow_low_precision(reason="bf16 ada"):
        nc.scalar.activation(semb[:], emb_sb[:], AF.Silu)
    scr = dram_pool.tile([4 * DIM], F32)
    with tc.tile_pool(name="ada_w", bufs=2) as awpool, \
         tc.tile_pool(name="ada_tmp", bufs=2) as atmp, \
         tc.tile_pool(name="ada_b", bufs=1) as abpool, \
         tc.tile_pool(name="ada_ps", bufs=2, space="PSUM") as app:
        b_t = abpool.tile([1, 4 * DIM], F32, tag="ada_bt")
        nc.sync.dma_start(b_t[:],
                          b_ap[:].rearrange("(o n) -> o n", o=1))
        for blk in range(ND):
            wt = awpool.tile([128, ND * 512], BF16, tag="adaw")
            nc.sync.dma_start(wt[:], w_ap[blk])
            ps = app.tile([1, 512], F32, tag="stat")
            co = 0 if blk < ND // 2 else ND
            for d in range(ND):
                nc.tensor.matmul(ps[:], semb[:, co + d:co + d + 1],
                                 wt[:, d * 512:(d + 1) * 512],
                                 start=(d == 0), stop=(d == ND - 1))
            ssb = atmp.tile([1, 512], F32, tag="ada_s")
            nc.vector.tensor_add(ssb[:], ps[:],
                                 b_t[:, blk * 512:(blk + 1) * 512])
            nc.sync.dma_start(scr[blk * 512:(blk + 1) * 512], ssb[:])
    out = []
    for idx in range(2):
        s2d = spool.tile([128, 2 * ND], F32, tag=f"s2d{idx}")
        nc.sync.dma_start(
            s2d[:], scr[idx * 2 * DIM:(idx + 1) * 2 * DIM]
            .rearrange("(j p) -> p j", p=128))
        onep = spool.tile([128, ND], F32, tag=f"onep{idx}")
        nc.vector.tensor_scalar_add(onep[:], s2d[:, 0:ND], 1.0)
        out += [s2d, onep]
    return out


def _layernorm(nc, tc, x_t, n, scale_fn, shift_fn, out_pool, out_tag,
               ones, eps_t, ones_stat=None, sq_dt=F32R):
    """Feature-major LN over len(x_t) tiles [128, n] + per-feature affine.
    Returns bf16 tiles. ones_stat must match the x/sq dtype."""
    nd = len(x_t)
    ones_col = ones_stat if ones_stat is not None \
        else ones[:, 0:1].bitcast(F32R)
    with tc.tile_pool(name="ln_s", bufs=1) as spool, \
         tc.tile_pool(name="ln_tmp", bufs=3) as tmp_pool, \
         tc.tile_pool(name="ln_ps", bufs=2, space="PSUM") as pp_stat, \
         tc.tile_pool(name="ln_bc", bufs=2, space="PSUM") as pp_bc:
        ps_sum = pp_stat.tile([1, n], F32, tag="stat")
        for j in range(nd):
            nc.tensor.matmul(ps_sum[:], ones_col, _r(x_t[j][:]),
                             start=(j == 0), stop=(j == nd - 1))
        ps_sq = pp_stat.tile([1, n], F32, tag="stat")
        for j in range(nd):
            sq = tmp_pool.tile([128, n], sq_dt, tag="ln_sq")
            with nc.allow_low_precision(reason="bf16 sq for LN stats"):
                nc.scalar.activation(sq[:], x_t[j][:], AF.Square)
            nc.tensor.matmul(ps_sq[:], ones_col, sq[:],
                             start=(j == 0), stop=(j == nd - 1))
        mean = spool.tile([1, n], F32R, tag="ln_mean")
        nc.scalar.activation(mean[:], ps_sum[:], AF.Copy,
                             scale=1.0 / (nd * 128))
        msq = spool.tile([1, n], F32, tag="ln_msq")
        nc.scalar.activation(msq[:], ps_sq[:], AF.Copy,
                             scale=1.0 / (nd * 128))
        m2 = spool.tile([1, n], F32, tag="ln_m2")
        nc.vector.tensor_mul(m2[:], mean[:], mean[:])
        var = spool.tile([1, n], F32, tag="ln_var")
        nc.vector.tensor_sub(var[:], msq[:], m2[:])
        std = spool.tile([1, n], F32, tag="ln_std")
        nc.scalar.activation(std[:], var[:], AF.Sqrt, bias=eps_t[:])
        rstd = spool.tile([1, n], F32R, tag="ln_rstd")
        with nc.allow_low_precision(reason="rstd feeds fp32r bcast matmul"):
            nc.vector.reciprocal(rstd[:], std[:])
        ps_mb = pp_bc.tile([128, n], F32, tag="bcast")
        nc.tensor.matmul(ps_mb[:], ones[0:1, :].bitcast(F32R), mean[:],
                         start=True, stop=True)
        ps_rb = pp_bc.tile([128, n], F32, tag="bcast")
        nc.tensor.matmul(ps_rb[:], ones[0:1, :].bitcast(F32R), rstd[:],
                         start=True, stop=True)
        h_t = []
        for j in range(nd):
            xc = tmp_pool.tile([128, n], F32, tag="ln_xc")
            nc.vector.tensor_sub(xc[:], x_t[j][:], ps_mb[:])
            xn = tmp_pool.tile([128, n], F32, tag="ln_xn")
            nc.vector.tensor_mul(xn[:], xc[:], ps_rb[:])
            h = out_pool.tile([128, n], BF16, tag=out_tag)
            with nc.allow_low_precision(reason="bf16 gemm operands"):
                nc.scalar.activation(h[:], xn[:], AF.Identity,
                                     bias=shift_fn(j), scale=scale_fn(j))
            h_t.append(h)
        return h_t


def _proj_mtiles(nc, wimg_pool, pp, img_ap, h_t, out_cb, tag, nm=NDP):
    """m-tile projection: out[m] = sum_d w[d,m]^T h[d]; out_cb(m, ps)."""
    for m in range(nm):
        wt = wimg_pool.tile([128, DIM], BF16, tag=tag)
        nc.sync.dma_start(wt[:], img_ap[m])
        ps = pp.tile([128, T], F32, tag="mm")
        for d in range(ND):
            nc.tensor.matmul(ps[:], wt[:, d * 128:(d + 1) * 128],
                             h_t[d][:], start=(d == 0), stop=(d == ND - 1))
        out_cb(m, ps)


def _out_proj(nc, pp, stage, xpool, wimg_pool, o_pk, wo_img, bias_col,
              x_prev_fn, x_tag):
    """Attn out-projection from packed o tiles + bias + residual."""
    x_new = []
    for m in range(ND):
        wt = wimg_pool.tile([128, NDP * 128], BF16, tag="wimg")
        nc.sync.dma_start(wt[:], wo_img[m])
        ps = pp.tile([128, T], F32, tag="mm")
        for b in range(NDP):
            nc.tensor.matmul(ps[:], wt[:, b * 128:(b + 1) * 128],
                             o_pk[b][:], start=(b == 0),
                             stop=(b == NDP - 1))
        t1 = stage.tile([128, T], F32, tag="t1")
        nc.scalar.activation(t1[:], ps[:], AF.Identity, bias=bias_col(m))
        xn = xpool.tile([128, T], F32R, tag=x_tag)
        with nc.allow_low_precision(reason="residual stream fp32r"):
            nc.vector.tensor_add(xn[:], t1[:], x_prev_fn(m))
        x_new.append(xn)
    return x_new


def _attn_core(nc, tc, pools, kslice_a, kslice_b, vslice_a, vslice_b,
               q_a, q_b, nkt, kpart, e_tag):
    """Shared attention inner loop: per head scores->exp->PV->normalize.
    Returns packed o tiles (8 a-blocks + 2 b-blocks), bf16."""
    (opool, epool, apool, pp, pp_pva, pp_pvb, pp_bc, ones) = pools
    o_pk = [opool.tile([128, T], BF16, tag=f"opk{i}", name=f"opk{i}")
            for i in range(8)]
    o_pkb = [opool.tile([128, T], BF16, tag=f"opkb{i}", name=f"opkb{i}")
             for i in range(3)]
    nc.vector.memset(o_pkb[0][96:128, :], 0.0)
    nc.vector.memset(o_pkb[1][96:128, :], 0.0)
    nc.vector.memset(o_pkb[2][64:128, :], 0.0)
    def normalize(h, ps_a, ps_b):
        rt = apool.tile([33, T], F32R, tag="recip")
        with nc.allow_low_precision(reason="softmax recip"):
            nc.vector.reciprocal(rt[32:33, :], ps_b[32:33, :])
        ps_rb = pp_bc.tile([128, T], F32, tag="bcast")
        nc.tensor.matmul(ps_rb[:], ones[32:33, :].bitcast(F32R),
                         rt[32:33, :], start=True, stop=True)
        rb = apool.tile([128, T], F32, tag="rb")
        nc.scalar.copy(rb[:], ps_rb[:])
        blk, r0 = _bslot(h)
        with nc.allow_low_precision(reason="attn out bf16"):
            nc.vector.tensor_mul(o_pk[h][:], ps_a[:], rb[:])
            nc.vector.tensor_mul(o_pkb[blk - 8][r0:r0 + 32, :],
                                 ps_b[0:32, :], rb[r0:r0 + 32, :])

    LAG = 4
    for h in range(HEADS):
        e_t = []
        ps_a = pp_pva.tile([128, T], F32, tag="pva")
        ps_b = pp_pvb.tile([128, T], F32, tag="pvb")

        def emit_pv(kt, ps_a=ps_a, ps_b=ps_b, e_t=e_t, h=h):
            nc.tensor.matmul(ps_a[:], vslice_a(h, kt), e_t[kt][:],
                             start=(kt == 0), stop=(kt == nkt - 1))
            nc.tensor.matmul(ps_b[:], vslice_b(h, kt), e_t[kt][:],
                             start=(kt == 0), stop=(kt == nkt - 1))

        for kt in range(nkt):
            ps = pp.tile([kpart, T], F32, tag="mm")
            nc.tensor.matmul(ps[:], kslice_a(h, kt), q_a(h),
                             start=True, stop=False)
            nc.tensor.matmul(ps[:], kslice_b(h, kt), q_b(h),
                             start=False, stop=True)
            ex = epool.tile([kpart, T], BF16, tag=e_tag)
            with nc.allow_low_precision(reason="bf16 probs"):
                nc.scalar.activation(ex[:], ps[:], AF.Exp, scale=ATT_SCALE)
            e_t.append(ex)
            if kt >= LAG:
                emit_pv(kt - LAG)
        for kt in range(max(0, nkt - LAG), nkt):
            emit_pv(kt)
        normalize(h, ps_a, ps_b)
    return o_pk + o_pkb


# --------------------------------------------------------------------------
# the single-launch program
# --------------------------------------------------------------------------

def _build():
    nc = bacc.Bacc("TRN2", target_bir_lowering=False, debug=False,
                   num_devices=NCORES)
    P = nc.declare_dram_parameter
    t = {}
    t["xT"] = P("xT", [DIM, T], BF16, isOutput=False)
    t["ctxT"] = P("ctxT", [CROSS, SCP], BF16, isOutput=False)
    t["emb_img"] = P("emb_img", [128, 2 * ND], F32, isOutput=False)
    t["ada_w_img"] = P("ada_w_img", [ND, 128, ND * 512], BF16,
                       isOutput=False)
    t["ada_b_img"] = P("ada_b_img", [4 * DIM], F32, isOutput=False)
    t["wq_img"] = P("wq_img", [NDP, 128, DIM], BF16, isOutput=False)
    t["wk_img"] = P("wk_img", [NDP, 128, DIM], BF16, isOutput=False)
    t["wv_img"] = P("wv_img", [ND, 128, VW], BF16, isOutput=False)
    t["wo1_img"] = P("wo1_img", [ND, 128, NDP * 128], BF16, isOutput=False)
    t["wq2_img"] = P("wq2_img", [NDP, 128, DIM], BF16, isOutput=False)
    t["wk2_img"] = P("wk2_img", [NDP, 128, CROSS], BF16, isOutput=False)
    t["wv2_img"] = P("wv2_img", [NDC, 128, VW], BF16, isOutput=False)
    t["wo2_img"] = P("wo2_img", [ND, 128, NDP * 128], BF16, isOutput=False)
    t["biases_img"] = P("biases_img", [128, 4 * ND], F32, isOutput=False)
    t["w1_img"] = P("w1_img", [NM1, 128, DIM], BF16, isOutput=False)
    t["b1_img"] = P("b1_img", [128, NM1], F32, isOutput=False)
    t["w2_img"] = P("w2_img", [ND, 128, INNER], BF16, isOutput=False)
    t["b2_img"] = P("b2_img", [128, ND], F32, isOutput=False)
    t["yT"] = P("yT", [DIM, T], F32, isOutput=True)

    with tile.TileContext(nc) as tc:
        _kernel_body(nc, tc, t)
    nc.compile()
    return nc


def _kernel_body(nc, tc, t):
    import contextlib
    with contextlib.ExitStack() as es:
        e = es.enter_context
        cpool = e(tc.tile_pool(name="const", bufs=1))
        spool = e(tc.tile_pool(name="spool", bufs=1))
        stage = e(tc.tile_pool(name="stage", bufs=2))
        xpool3 = e(tc.tile_pool(name="xp3", bufs=ND))
        dram_pool = e(tc.tile_pool(name="dram", bufs=1, space="DRAM"))

        ones, ones_bf, eps_t = _consts(nc, cpool)
        biases = cpool.tile([128, 4 * ND], F32, tag="biases")
        nc.sync.dma_start(biases[:], t["biases_img"][:])
        b1_t = cpool.tile([128, NM1], F32, tag="b1")
        nc.sync.dma_start(b1_t[:], t["b1_img"][:])
        b2_t = cpool.tile([128, ND], F32, tag="b2")
        nc.sync.dma_start(b2_t[:], t["b2_img"][:])

        x0 = cpool.tile([128, ND * T], BF16, tag="x0")
        nc.sync.dma_start(x0[:].rearrange("p (b t) -> p b t", t=T),
                          t["xT"][:].rearrange("(b p) t -> p b t", p=128))
        x_t = [x0[:, j * T:(j + 1) * T] for j in range(ND)]

        # ---------------- cross-attn context K/V (independent; early) ------
        ctx_sb = cpool.tile([128, NDC * SCP], BF16, tag="ctx")
        nc.sync.dma_start(ctx_sb[:].rearrange("p (d s) -> p d s", s=SCP),
                          t["ctxT"][:].rearrange("(d p) s -> p d s", p=128))
        ctx_t = [ctx_sb[:, d * SCP:(d + 1) * SCP] for d in range(NDC)]
        # ---------------- adaLN embeddings ----------------
        s2d1, onep1, s2d2, onep2 = _ada_local(
            nc, tc, t["emb_img"], t["ada_w_img"], t["ada_b_img"],
            spool, dram_pool)

        KSP = 6                       # K AG split: blocks 0-5 / 6-10
        kstg = [dram_pool.tile([KSP * 128, T], BF16, name="kstg0"),
                dram_pool.tile([(NDP - KSP) * 128, T], BF16, name="kstg1")]
        vstg = [dram_pool.tile([2 * 128, VW], BF16, name="vstg0"),
                dram_pool.tile([2 * 128, VW], BF16, name="vstg1")]
        kgth = [dram_pool.tile([GROUP, KSP * 128, T], BF16, name="kgth0"),
                dram_pool.tile([GROUP, (NDP - KSP) * 128, T], BF16,
                               name="kgth1")]
        vgth = [dram_pool.tile([GROUP, 2 * 128, VW], BF16, name="vgth0"),
                dram_pool.tile([GROUP, 2 * 128, VW], BF16, name="vgth1")]
        groups = [[0, 1, 2, 3], [4, 5, 6, 7]]

        # ---------------- LN1 + q/k/v projections ----------------
        xp2_es = contextlib.ExitStack()
        xpool2 = xp2_es.enter_context(tc.tile_pool(name="xp2", bufs=ND))
        kvq_es = contextlib.ExitStack()
        q_sb = kvq_es.enter_context(
            tc.tile_pool(name="qp", bufs=1)).tile(
            [128, NDP * T], BF16, tag="q")
        with tc.tile_pool(name="hp", bufs=ND) as hpool, \
             tc.tile_pool(name="wimg", bufs=3) as wimg, \
             tc.tile_pool(name="kv1", bufs=1) as kv1pool, \
             tc.tile_pool(name="kv", bufs=2) as kvpool, \
             tc.tile_pool(name="ppa", bufs=3, space="PSUM") as pp:
            h_t = _layernorm(nc, tc, x_t, T,
                             lambda j: onep1[:, j:j + 1],
                             lambda j: s2d1[:, ND + j:ND + j + 1],
                             hpool, "h", ones, eps_t,
                             ones_stat=ones_bf[:, 0:1], sq_dt=BF16)

            k_all = kv1pool.tile([128, NDP * T], BF16, tag="kall")

            def k_out(m, ps):
                with nc.allow_low_precision(reason="bf16 k"):
                    nc.vector.tensor_copy(k_all[:, m * T:(m + 1) * T],
                                          ps[:])

            for m in range(NDP):
                wt = wimg.tile([128, DIM], BF16, tag="wimg")
                nc.sync.dma_start(wt[:], t["wk_img"][m])
                ps = pp.tile([128, T], F32, tag="mm")
                for d in range(ND):
                    nc.tensor.matmul(ps[:], wt[:, d * 128:(d + 1) * 128],
                                     h_t[d][:], start=(d == 0),
                                     stop=(d == ND - 1))
                k_out(m, ps)
                if m == KSP - 1 or m == NDP - 1:
                    half = 0 if m == KSP - 1 else 1
                    c0 = 0 if half == 0 else KSP * T
                    nb = KSP if half == 0 else NDP - KSP
                    nc.sync.dma_start(
                        kstg[half][:].rearrange("(b p) t -> p b t", p=128),
                        k_all[:, c0:c0 + nb * T]
                        .rearrange("p (b t) -> p b t", t=T))
                    nc.gpsimd.collective_compute(
                        "AllGather", mybir.AluOpType.bypass,
                        replica_groups=groups, ins=[kstg[half][:]],
                        outs=[kgth[half][:]])

            def q_out(m, ps):
                with nc.allow_low_precision(reason="bf16 q"):
                    nc.vector.tensor_copy(q_sb[:, m * T:(m + 1) * T],
                                          ps[:])

            _proj_mtiles(nc, wimg, pp, t["wq_img"], h_t, q_out, "wimg")
            wv_sb = kv1pool.tile([128, ND * VW], BF16, tag="wv")
            nc.sync.dma_start(
                wv_sb[:].rearrange("p (d c) -> p d c", c=VW),
                t["wv_img"][:].rearrange("d p c -> p d c"))
            for tt in range(NTT):
                vtile = kvpool.tile([128, VW], BF16, tag="vtile")
                for off, nn in ((0, 512), (512, 512), (1024, VW - 1024)):
                    ps = pp.tile([128, 512], F32, tag="mm")
                    for d in range(ND):
                        nc.tensor.matmul(
                            ps[:, 0:nn], h_t[d][:, tt * 128:(tt + 1) * 128],
                            wv_sb[:, d * VW + off:d * VW + off + nn],
                            start=(d == 0), stop=(d == ND - 1))
                    with nc.allow_low_precision(reason="bf16 v"):
                        nc.scalar.copy(vtile[:, off:off + nn], ps[:, 0:nn])
                for h in range(HEADS):
                    nc.vector.memset(vtile[:, h * DHP + DH:(h + 1) * DHP],
                                     1.0)
                half, r = tt // 2, (tt % 2) * 128
                nc.sync.dma_start(vstg[half][r:r + 128, :], vtile[:])
                if tt % 2 == 1:
                    nc.gpsimd.collective_compute(
                        "AllGather", mybir.AluOpType.bypass,
                        replica_groups=groups, ins=[vstg[half][:]],
                        outs=[vgth[half][:]])


        # ---------------- cross-attn context K/V (fills the AG gap) --
        k2_sb = cpool.tile([128, NDP * SCP], BF16, tag="k2")
        v2pad = cpool.tile([SCP, VW + 128], BF16, tag="v2pad")
        nc.vector.memset(v2pad[:], 0.0)
        with tc.tile_pool(name="cw", bufs=3) as cwpool, \
             tc.tile_pool(name="cwv1", bufs=1) as cwvpool, \
             tc.tile_pool(name="cps", bufs=2, space="PSUM") as cpp:
            for m in range(NDP):
                wt = cwpool.tile([128, CROSS], BF16, tag="cw")
                nc.sync.dma_start(wt[:], t["wk2_img"][m])
                ps = cpp.tile([128, SCP], F32, tag="cmm")
                for d in range(NDC):
                    nc.tensor.matmul(ps[:], wt[:, d * 128:(d + 1) * 128],
                                     ctx_t[d], start=(d == 0),
                                     stop=(d == NDC - 1))
                with nc.allow_low_precision(reason="bf16 k2"):
                    nc.scalar.copy(k2_sb[:, m * SCP:(m + 1) * SCP], ps[:])
            wv2_sb = cwvpool.tile([128, NDC * VW], BF16, tag="cwv")
            nc.sync.dma_start(
                wv2_sb[:].rearrange("p (d c) -> p d c", c=VW),
                t["wv2_img"][:].rearrange("d p c -> p d c"))
            for off, nn in ((0, 512), (512, 512), (1024, VW - 1024)):
                ps = cpp.tile([SCP, 512], F32, tag="cmm2")
                for d in range(NDC):
                    nc.tensor.matmul(ps[:, 0:nn], ctx_t[d],
                                     wv2_sb[:, d * VW + off:d * VW + off + nn],
                                     start=(d == 0), stop=(d == NDC - 1))
                with nc.allow_low_precision(reason="bf16 v2"):
                    nc.scalar.copy(v2pad[:, off:off + nn], ps[:, 0:nn])
            # ones columns for the denominator (rows 77:80 stay zero: ctx
            # padding is zero so the psum wrote zeros there, and the host
            # zeroed the wv2 ones-columns)
            for h in range(HEADS):
                nc.vector.memset(v2pad[0:SCTX, h * DHP + DH:(h + 1) * DHP],
                                 1.0)

        # ---------------- self-attention ----------------
        with tc.tile_pool(name="att_o", bufs=1) as opool, \
             tc.tile_pool(name="att_e", bufs=10) as epool, \
             tc.tile_pool(name="att_s", bufs=2) as apool, \
             tc.tile_pool(name="wimg", bufs=3) as wimg, \
             tc.tile_pool(name="ppa", bufs=3, space="PSUM") as pp, \
             tc.tile_pool(name="ppv", bufs=2, space="PSUM") as pp_pva, \
             tc.tile_pool(name="ppw", bufs=2, space="PSUM") as pp_pvb, \
             tc.tile_pool(name="ppb", bufs=1, space="PSUM") as pp_bc:
            gkv_es = contextlib.ExitStack()
            gkvpool = gkv_es.enter_context(tc.tile_pool(name="gkv", bufs=1))
            kt_sb = [gkvpool.tile([128, NDP * T], BF16, tag=f"ktg{g}",
                                  name=f"ktg{g}") for g in range(GROUP)]
            vt_sb = [gkvpool.tile([128, NTT * VW + 128], BF16,
                                  tag=f"vtg{g}", name=f"vtg{g}")
                     for g in range(GROUP)]
            for g in range(GROUP):
                nc.vector.memset(vt_sb[g][:, NTT * VW:], 0.0)
            for g in range(GROUP):
                nc.sync.dma_start(
                    kt_sb[g][:, 0:KSP * T]
                    .rearrange("p (b t) -> p b t", t=T),
                    kgth[0][g].rearrange("(b p) t -> p b t", p=128))
                nc.sync.dma_start(
                    kt_sb[g][:, KSP * T:]
                    .rearrange("p (b t) -> p b t", t=T),
                    kgth[1][g].rearrange("(b p) t -> p b t", p=128))
            for g in range(GROUP):
                nc.sync.dma_start(
                    vt_sb[g][:, 0:2 * VW]
                    .rearrange("p (j c) -> p j c", c=VW),
                    vgth[0][g].rearrange("(j p) c -> p j c", p=128))
                nc.sync.dma_start(
                    vt_sb[g][:, 2 * VW:4 * VW]
                    .rearrange("p (j c) -> p j c", c=VW),
                    vgth[1][g].rearrange("(j p) c -> p j c", p=128))

            def kslice_a(h, kt):
                g, j = kt // NTT, kt % NTT
                return kt_sb[g][:, h * T + j * 128:h * T + (j + 1) * 128]

            qb_pad = gkvpool.tile([128, HEADS * T], BF16, tag="qbp")
            nc.vector.memset(qb_pad[:], 0.0)
            for h in range(HEADS):
                blk, r0 = _bslot(h)
                with nc.allow_low_precision(reason="bf16 qb"):
                    nc.vector.tensor_copy(
                        qb_pad[r0:r0 + 32, h * T:(h + 1) * T],
                        q_sb[r0:r0 + 32, blk * T:(blk + 1) * T])

            def kslice_b(h, kt):
                g, j = kt // NTT, kt % NTT
                blk, r0 = _bslot(h)
                return kt_sb[g][:, blk * T + j * 128:blk * T + (j + 1) * 128]

            def vslice_a(h, kt):
                g, j = kt // NTT, kt % NTT
                return vt_sb[g][:, j * VW + h * DHP:j * VW + h * DHP + 128]

            def vslice_b(h, kt):
                g, j = kt // NTT, kt % NTT
                return vt_sb[g][:, j * VW + h * DHP + 128:
                                j * VW + h * DHP + 256]

            def q_a(h):
                return q_sb[:, h * T:(h + 1) * T]

            def q_b(h):
                return qb_pad[:, h * T:(h + 1) * T]

            pools = (opool, epool, apool, pp, pp_pva, pp_pvb, pp_bc, ones)
            o_pk = _attn_core(nc, tc, pools, kslice_a, kslice_b,
                              vslice_a, vslice_b, q_a, q_b, NKT, 128, "e1")
            gkv_es.close()
            x2_t = _out_proj(nc, pp, stage, xpool2, wimg, o_pk,
                             t["wo1_img"],
                             lambda m: biases[:, m:m + 1],
                             lambda m: x_t[m], "x2")
        kvq_es.close()

        # ---------------- cross-attention ----------------
        q2_es = contextlib.ExitStack()
        q2_sb = q2_es.enter_context(
            tc.tile_pool(name="q2p", bufs=1)).tile(
            [128, NDP * T], BF16, tag="q2")
        with tc.tile_pool(name="hp", bufs=ND) as hpool, \
             tc.tile_pool(name="wimg", bufs=3) as wimg, \
             tc.tile_pool(name="cr_o", bufs=1) as opool, \
             tc.tile_pool(name="cr_e", bufs=4) as epool, \
             tc.tile_pool(name="cr_s", bufs=2) as apool:
            h2_t = _layernorm(nc, tc, x2_t, T,
                              lambda j: onep2[:, j:j + 1],
                              lambda j: s2d2[:, ND + j:ND + j + 1],
                              hpool, "h", ones, eps_t)

            def q2_out(m, ps):
                with nc.allow_low_precision(reason="bf16 q2"):
                    nc.vector.tensor_copy(q2_sb[:, m * T:(m + 1) * T],
                                          ps[:])

            with tc.tile_pool(name="ppq2", bufs=3, space="PSUM") as ppq2:
                _proj_mtiles(nc, wimg, ppq2, t["wq2_img"], h2_t, q2_out,
                             "wimg")

            def k2slice_a(h, kt):
                return k2_sb[:, h * SCP:(h + 1) * SCP]

            q2b_pad = opool.tile([128, HEADS * T], BF16, tag="q2bp")
            nc.vector.memset(q2b_pad[:], 0.0)
            for h in range(HEADS):
                blk, r0 = _bslot(h)
                with nc.allow_low_precision(reason="bf16 q2b"):
                    nc.vector.tensor_copy(
                        q2b_pad[r0:r0 + 32, h * T:(h + 1) * T],
                        q2_sb[r0:r0 + 32, blk * T:(blk + 1) * T])

            def k2slice_b(h, kt):
                blk, r0 = _bslot(h)
                return k2_sb[:, blk * SCP:(blk + 1) * SCP]

            def v2slice_a(h, kt):
                return v2pad[:, h * DHP:h * DHP + 128]

            def v2slice_b(h, kt):
                return v2pad[:, h * DHP + 128:h * DHP + 256]

            def q2_a(h):
                return q2_sb[:, h * T:(h + 1) * T]

            def q2_b(h):
                return q2b_pad[:, h * T:(h + 1) * T]

            with tc.tile_pool(name="ppa", bufs=2, space="PSUM") as pp, \
                 tc.tile_pool(name="ppv", bufs=2, space="PSUM") as pp_pva, \
                 tc.tile_pool(name="ppw", bufs=2, space="PSUM") as pp_pvb, \
                 tc.tile_pool(name="ppb", bufs=1, space="PSUM") as pp_bc:
                pools = (opool, epool, apool, pp, pp_pva, pp_pvb, pp_bc,
                         ones)
                o2_pk = _attn_core(nc, tc, pools, k2slice_a, k2slice_b,
                                   v2slice_a, v2slice_b, q2_a, q2_b, 1, SCP,
                                   "e2")
                x3_t = _out_proj(nc, pp, stage, xpool3, wimg, o2_pk,
                                 t["wo2_img"],
                                 lambda m: biases[:, ND + m:ND + m + 1],
                                 lambda m: x2_t[m][:], "x3")
        q2_es.close()
        xp2_es.close()

        # ---------------- GEGLU feed-forward ----------------
        with tc.tile_pool(name="hp", bufs=ND) as hpool, \
             tc.tile_pool(name="wimg", bufs=4) as wimg, \
             tc.tile_pool(name="ff_hg", bufs=NI) as hgpool, \
             tc.tile_pool(name="ff_u", bufs=3) as upool, \
             tc.tile_pool(name="ff_w2", bufs=2) as w2pool, \
             tc.tile_pool(name="ppa", bufs=4, space="PSUM") as pp:
            h3_t = _layernorm(nc, tc, x3_t, T,
                              lambda j: biases[:, 2 * ND + j:2 * ND + j + 1],
                              lambda j: biases[:, 3 * ND + j:3 * ND + j + 1],
                              hpool, "h", ones, eps_t)
            hg_t = []
            for i in range(NI):
                wt = wimg.tile([128, 2 * DIM], BF16, tag="w1pair")
                nc.sync.dma_start(
                    wt[:].rearrange("p (i c) -> p i c", c=DIM),
                    t["w1_img"][2 * i:2 * i + 2].rearrange("i p c -> p i c"))
                ps_u = pp.tile([128, T], F32, tag="mm")
                for d in range(ND):
                    nc.tensor.matmul(ps_u[:], wt[:, d * 128:(d + 1) * 128],
                                     h3_t[d][:], start=(d == 0),
                                     stop=(d == ND - 1))
                ps_g = pp.tile([128, T], F32, tag="mm")
                for d in range(ND):
                    nc.tensor.matmul(ps_g[:],
                                     wt[:, DIM + d * 128:DIM + (d + 1) * 128],
                                     h3_t[d][:], start=(d == 0),
                                     stop=(d == ND - 1))
                u = upool.tile([128, T], F32, tag="u")
                nc.scalar.activation(u[:], ps_u[:], AF.Identity,
                                     bias=b1_t[:, 2 * i:2 * i + 1])
                g = upool.tile([128, T], F32, tag="g")
                nc.scalar.activation(g[:], ps_g[:], AF.Gelu,
                                     bias=b1_t[:, 2 * i + 1:2 * i + 2])
                hg = hgpool.tile([128, T], BF16, tag="hg")
                with nc.allow_low_precision(reason="bf16 geglu"):
                    nc.vector.tensor_mul(hg[:], u[:], g[:])
                hg_t.append(hg)
            for m in range(ND):
                wt = w2pool.tile([128, INNER], BF16, tag="w2")
                nc.sync.dma_start(wt[:], t["w2_img"][m])
                ps = pp.tile([128, T], F32, tag="mm")
                for d in range(NI):
                    nc.tensor.matmul(ps[:], wt[:, d * 128:(d + 1) * 128],
                                     hg_t[d][:], start=(d == 0),
                                     stop=(d == NI - 1))
                t1 = stage.tile([128, T], F32, tag="t1")
                nc.scalar.activation(t1[:], ps[:], AF.Identity,
                                     bias=b2_t[:, m:m + 1])
                y = stage.tile([128, T], F32, tag="y")
                nc.vector.tensor_add(y[:], t1[:], x3_t[m][:])
                nc.sync.dma_start(t["yT"][m * 128:(m + 1) * 128, :], y[:])


# --------------------------------------------------------------------------
# host side: weight images
# --------------------------------------------------------------------------

def _head_perm_pad():
    """Padded column map [NDP*128]: 8 blocks of head dims 0-127, then 3
    blocks packing per-head 32-dim remainders at offsets 0/32/64; -1 = zero
    padding."""
    idx = np.full(NDP * 128, -1, np.int64)
    for h in range(HEADS):
        idx[h * 128:(h + 1) * 128] = np.arange(h * DH, h * DH + 128)
    for h in range(HEADS):
        blk, off = _bslot(h)
        idx[blk * 128 + off:blk * 128 + off + 32] = np.arange(
            h * DH + 128, h * DH + DH)
    return idx


def _pad_cols(w, idx):
    out = np.zeros((w.shape[0], len(idx)), w.dtype)
    sel = idx >= 0
    out[:, sel] = w[:, idx[sel]]
    return out


def _pad_rows(w, idx):
    out = np.zeros((len(idx), w.shape[1]), w.dtype)
    sel = idx >= 0
    out[sel, :] = w[idx[sel], :]
    return out


def _img_kxm(w, mcols=128):
    """[K, M] weight -> [M//mcols, 128, (K//128)*mcols] m-tile images."""
    K, M = w.shape
    nd, nm = K // 128, M // mcols
    return np.ascontiguousarray(
        w.reshape(nd, 128, nm, mcols).transpose(2, 1, 0, 3)
        .reshape(nm, 128, nd * mcols))


def _v_pad(w):
    """[K, DIM] -> [K, VW] with a zero column after each head's 160 dims."""
    K = w.shape[0]
    out = np.zeros((K, VW), w.dtype)
    for h in range(HEADS):
        out[:, h * DHP:h * DHP + DH] = w[:, h * DH:(h + 1) * DH]
    return out


def _col_img(v):
    """[N] -> [128, N//128] image: img[p, j] = v[j*128 + p]."""
    return np.ascontiguousarray(v.reshape(-1, 128).T)


_STATE = {}

_STATIC_NAMES = (
    "ada_w_img", "ada_b_img", "wq_img", "wk_img", "wv_img", "wo1_img",
    "wq2_img", "wk2_img", "wv2_img", "wo2_img", "biases_img",
    "w1_img", "b1_img", "w2_img", "b2_img",
)


def _prepare(inputs):
    key = tuple(np.asarray(inputs[k]).ctypes.data for k in
                ("a1_wq", "ff_w1", "ff_w2", "a2_wk", "a1_wo"))
    if _STATE.get("key") == key:
        return _STATE["prep"]
    f = np.float32
    idx = _head_perm_pad()
    g = {}

    def bimg(w):
        return _img_kxm(np.asarray(w, f)).astype(NPBF)

    g["wq_img"] = bimg(_pad_cols(np.asarray(inputs["a1_wq"], f), idx))
    g["wk_img"] = bimg(_pad_cols(np.asarray(inputs["a1_wk"], f), idx))
    g["wv_img"] = np.ascontiguousarray(
        _v_pad(np.asarray(inputs["a1_wv"], f)).reshape(ND, 128, VW)
    ).astype(NPBF)
    g["wo1_img"] = bimg(_pad_rows(np.asarray(inputs["a1_wo"], f), idx))
    g["wq2_img"] = bimg(_pad_cols(np.asarray(inputs["a2_wq"], f), idx))
    g["wk2_img"] = bimg(_pad_cols(np.asarray(inputs["a2_wk"], f), idx))
    g["wv2_img"] = np.ascontiguousarray(
        _v_pad(np.asarray(inputs["a2_wv"], f)).reshape(NDC, 128, VW)
    ).astype(NPBF)
    g["wo2_img"] = bimg(_pad_rows(np.asarray(inputs["a2_wo"], f), idx))

    a1w = np.asarray(inputs["ada1_w"], f)
    a1b = np.asarray(inputs["ada1_b"], f)
    a2w = np.asarray(inputs["ada2_w"], f)
    a2b = np.asarray(inputs["ada2_b"], f)
    g["ada_w_img"] = _img_kxm(np.hstack([a1w, a2w]),
                              mcols=512).astype(NPBF)
    g["ada_b_img"] = np.ascontiguousarray(np.concatenate([a1b, a2b]))

    g["biases_img"] = np.ascontiguousarray(np.concatenate([
        _col_img(np.asarray(inputs["a1_bo"], f)),
        _col_img(np.asarray(inputs["a2_bo"], f)),
        _col_img(np.asarray(inputs["norm3_g"], f)),
        _col_img(np.asarray(inputs["norm3_b"], f))], axis=1))

    w1 = np.asarray(inputs["ff_w1"], f)
    w1i = np.stack([w1[:, :INNER].reshape(DIM, NI, 128),
                    w1[:, INNER:].reshape(DIM, NI, 128)],
                   axis=2).reshape(DIM, 2 * INNER)
    g["w1_img"] = bimg(w1i)
    b1 = np.asarray(inputs["ff_b1"], f)
    b1i = np.stack([b1[:INNER].reshape(NI, 128),
                    b1[INNER:].reshape(NI, 128)], axis=1).reshape(-1)
    g["b1_img"] = _col_img(b1i)
    g["w2_img"] = bimg(np.asarray(inputs["ff_w2"], f))
    g["b2_img"] = _col_img(np.asarray(inputs["ff_b2"], f))
    _STATE["key"] = key
    _STATE["prep"] = g
    _STATE.pop("static_dev", None)
    return g


# --------------------------------------------------------------------------
# SPMD runner with device-resident static inputs
# --------------------------------------------------------------------------

class _SpmdRunner:
    """Caches the jitted callable and keeps device-resident global arrays
    for static inputs."""

    def __init__(self, nc, n_cores):
        import functools

        import jax
        import jax.numpy as jnp
        from jax.experimental.shard_map import shard_map
        from jax.sharding import Mesh, NamedSharding, PartitionSpec

        from concourse import bass2jax

        bass2jax.install_neuronx_cc_hook()
        self.jax = jax
        self.nc = nc
        self.n_cores = n_cores
        partition_name = (nc.partition_id_tensor.name
                          if nc.partition_id_tensor else None)
        in_names, out_names, out_avals, zero_shapes = [], [], [], []
        for alloc in nc.m.functions[0].allocations:
            if not isinstance(alloc, mybir.MemoryLocationSet):
                continue
            name = alloc.memorylocations[0].name
            if alloc.kind == "ExternalInput":
                if name != partition_name:
                    in_names.append(name)
            elif alloc.kind == "ExternalOutput":
                shape = tuple(alloc.tensor_shape)
                dtype = mybir.dt.np(alloc.dtype)
                out_names.append(name)
                out_avals.append(jax.core.ShapedArray(shape, dtype))
                zero_shapes.append((shape, dtype))
        self.n_params = len(in_names)
        self.in_names = list(in_names)
        self.out_names = list(out_names)
        self.out_avals = out_avals
        all_in_names = list(in_names) + list(out_names)
        if partition_name is not None:
            all_in_names.append(partition_name)
        donate = tuple(range(self.n_params,
                             self.n_params + len(out_names)))

        def _bdy(*args):
            operands = list(args)
            if partition_name is not None:
                operands.append(bass2jax.partition_id_tensor())
            outs = bass2jax._bass_exec_p.bind(
                *operands,
                out_avals=tuple(out_avals),
                in_names=tuple(all_in_names),
                out_names=tuple(out_names),
                lowering_input_output_aliases=(),
                sim_require_finite=True,
                sim_require_nnan=True,
                nc=nc,
            )
            return tuple(outs)

        devices = jax.devices()[:n_cores]
        self.mesh = Mesh(np.asarray(devices), ("core",))
        self.sharding = NamedSharding(self.mesh, PartitionSpec("core"))
        n_z = len(zero_shapes)
        self.sharded = jax.jit(
            shard_map(_bdy, mesh=self.mesh,
                      in_specs=(PartitionSpec("core"),) * (self.n_params
                                                           + n_z),
                      out_specs=(PartitionSpec("core"),) * len(out_names),
                      check_rep=False),
            donate_argnums=donate, keep_unused=True)
        self._zero_fns = []
        for shape, dtype in zero_shapes:
            gshape = (n_cores * shape[0],) + tuple(shape[1:])
            self._zero_fns.append(jax.jit(
                functools.partial(jnp.zeros, gshape, dtype),
                out_shardings=self.sharding))
        self._static_cache = {}

    def put_static(self, name, per_core_arrays):
        gl = np.concatenate(per_core_arrays, axis=0)
        self._static_cache[name] = self.jax.device_put(gl, self.sharding)

    def __call__(self, in_maps):
        args = []
        for name in self.in_names:
            if name in self._static_cache:
                args.append(self._static_cache[name])
            else:
                gl = np.concatenate(
                    [np.asarray(m[name]) for m in in_maps], axis=0)
                args.append(self.jax.device_put(gl, self.sharding))
        zeros = [zf() for zf in self._zero_fns]
        out_arrs = self.sharded(*args, *zeros)
        res = []
        for c in range(self.n_cores):
            res.append({
                name: np.asarray(out_arrs[i]).reshape(
                    self.n_cores, *self.out_avals[i].shape)[c]
                for i, name in enumerate(self.out_names)})
        return res


def kernel(**inputs):
    if "nc" not in _STATE:
        _STATE["nc"] = _build()
    g = _prepare(inputs)
    if "runner" not in _STATE:
        _STATE["runner"] = _SpmdRunner(_STATE["nc"], NCORES)
    runner = _STATE["runner"]
    if "static_dev" not in _STATE:
        for name in _STATIC_NAMES:
            arrs = g[name]
            if not isinstance(arrs, list):
                arrs = [arrs] * NCORES
            runner.put_static(name, arrs)
        _STATE["static_dev"] = True
    f = np.float32
    x = np.asarray(inputs["hidden_states"], f)
    ctx = np.asarray(inputs["context"], f)
    tstep = int(np.asarray(inputs["timestep"]))
    emb_img = np.ascontiguousarray(np.concatenate(
        [_col_img(np.asarray(inputs["ada1_emb"], f)[tstep]),
         _col_img(np.asarray(inputs["ada2_emb"], f)[tstep])], axis=1))

    xT = [np.ascontiguousarray(x[b].T) for b in range(B)]  # [DIM, S]
    xT_c = [np.ascontiguousarray(xT[c // GROUP][:, (c % GROUP) * T:
                                                (c % GROUP + 1) * T]
                                 ).astype(NPBF)
            for c in range(NCORES)]
    ctxT = []
    for b in range(B):
        cp = np.zeros((CROSS, SCP), f)
        cp[:, :SCTX] = ctx[b].T
        ctxT.append(cp.astype(NPBF))

    in_maps = [{
        "xT": xT_c[c], "ctxT": ctxT[c // GROUP], "emb_img": emb_img,
    } for c in range(NCORES)]
    res = runner(in_maps)

    y = np.empty((B, S, DIM), f)
    for c in range(NCORES):
        b, i = divmod(c, GROUP)
        y[b, i * T:(i + 1) * T, :] = res[c]["yT"].T
    return y


# revision 25
# speedup vs baseline: 1.1373x; 1.0409x over previous
"""Trainium2 Bass kernel for a BasicTransformerBlock (self-attn + cross-attn + GEGLU FF).

Sharding: sequence-parallel over the 8 cores. Core c handles batch b=c//4,
token chunk (c%4)*512 : (c%4+1)*512, feature-major [D, T] on device.
K/V for the full batch are exchanged with an on-device AllGather per 4-core
group. All GEMM operands are bf16 (PSUM accumulation stays fp32); the
residual stream is fp32. Projection outputs that feed attention use a
head-permuted column order (8 blocks of dims 0-127 per head, then 2 blocks
packing the 32-dim per-head remainders) so every attention matmul is a slice
of a few large SBUF tiles, and V carries a ones column per head so the
softmax denominator rides along in the PV matmul.
"""
import sys

import numpy as np

sys.path.insert(0, "/opt/trn_rl_repo")

import ml_dtypes  # noqa: E402

import concourse.bass as bass  # noqa: E402
import concourse.tile as tile  # noqa: E402
from concourse import bacc, mybir  # noqa: E402

F32 = mybir.dt.float32
F32R = mybir.dt.float32r
BF16 = mybir.dt.bfloat16
NPBF = ml_dtypes.bfloat16
AF = mybir.ActivationFunctionType

B, S, DIM, SCTX, CROSS, INNER = 2, 2048, 1280, 77, 768, 5120
HEADS, DH = 8, 160
NCORES = 8
T = (B * S) // NCORES          # 512 tokens per core
GROUP = NCORES // B            # 4 cores per batch
ND = DIM // 128                # 10
NDC = CROSS // 128             # 6
NKT = S // 128                 # 16
NM1 = (2 * INNER) // 128       # 80
NI = INNER // 128              # 40
LN_EPS = 1e-5
ATT_SCALE = DH ** -0.5
DHP = DH + 1                   # v column group padded with a ones column
VW = HEADS * DHP               # 1288
SCP = 80                       # context tokens padded 77 -> 80
NTT = T // 128                 # 4 token tiles per core
NDP = 11                       # head-packed blocks: 8 main + 3 remainder


def _r(ap):
    return ap if ap.dtype in (F32R, BF16) else ap.bitcast(F32R)


def _bslot(h):
    """(block, partition offset) of head h's 32-dim remainder; offsets are
    limited to {0, 32, 64} by the PE base-partition constraint."""
    return 8 + h // 3, (h % 3) * 32


# --------------------------------------------------------------------------
# device-side building blocks
# --------------------------------------------------------------------------

def _consts(nc, cpool):
    ones = cpool.tile([128, 128], F32, tag="ones")
    nc.any.memset(ones[:], 1.0)
    ones_bf = cpool.tile([128, 1], BF16, tag="ones_bf")
    nc.any.memset(ones_bf[:], 1.0)
    eps_t = cpool.tile([1, 1], F32, tag="eps")
    nc.any.memset(eps_t[:], LN_EPS)
    return ones, ones_bf, eps_t


ADASL = (4 * DIM) // NCORES    # 640 output cols of one ada per core


def _ada_local(nc, tc, emb_ap, w_ap, b_ap, spool, dram_pool):
    """Both adaLN embeddings computed locally on every core (no collective).
    Returns (s2d1, onep1, s2d2, onep2) as [128, 2*ND]/[128, ND] images."""
    emb_sb = spool.tile([128, 2 * ND], F32, tag="emb_sb")
    nc.sync.dma_start(emb_sb[:], emb_ap[:])
    semb = spool.tile([128, 2 * ND], BF16, tag="semb")
    with nc.allow_low_precision(reason="bf16 ada"):
        nc.scalar.activation(semb[:], emb_sb[:], AF.Silu)
    scr = dram_pool.tile([4 * DIM], F32)
    with tc.tile_pool(name="ada_w", bufs=2) as awpool, \
         tc.tile_pool(name="ada_tmp", bufs=2) as atmp, \
         tc.tile_pool(name="ada_b", bufs=1) as abpool, \
         tc.tile_pool(name="ada_ps", bufs=2, space="PSUM") as app:
        b_t = abpool.tile([1, 4 * DIM], F32, tag="ada_bt")
        nc.sync.dma_start(b_t[:],
                          b_ap[:].rearrange("(o n) -> o n", o=1))
        for blk in range(ND):
            wt = awpool.tile([128, ND * 512], BF16, tag="adaw")
            nc.sync.dma_start(wt[:], w_ap[blk])
            ps = app.tile([1, 512], F32, tag="stat")
            co = 0 if blk < ND // 2 else ND
            for d in range(ND):
                nc.tensor.matmul(ps[:], semb[:, co + d:co + d + 1],
                                 wt[:, d * 512:(d + 1) * 512],
                                 start=(d == 0), stop=(d == ND - 1))
            ssb = atmp.tile([1, 512], F32, tag="ada_s")
            nc.vector.tensor_add(ssb[:], ps[:],
                                 b_t[:, blk * 512:(blk + 1) * 512])
            nc.sync.dma_start(scr[blk * 512:(blk + 1) * 512], ssb[:])
    out = []
    for idx in range(2):
        s2d = spool.tile([128, 2 * ND], F32, tag=f"s2d{idx}")
        nc.sync.dma_start(
            s2d[:], scr[idx * 2 * DIM:(idx + 1) * 2 * DIM]
            .rearrange("(j p) -> p j", p=128))
        onep = spool.tile([128, ND], F32, tag=f"onep{idx}")
        nc.vector.tensor_scalar_add(onep[:], s2d[:, 0:ND], 1.0)
        out += [s2d, onep]
    return out


def _layernorm(nc, tc, x_t, n, scale_fn, shift_fn, out_pool, out_tag,
               ones, eps_t, ones_stat=None, sq_dt=F32R):
    """Feature-major LN over len(x_t) tiles [128, n] + per-feature affine.
    Returns bf16 tiles. ones_stat must match the x/sq dtype."""
    nd = len(x_t)
    ones_col = ones_stat if ones_stat is not None \
        else ones[:, 0:1].bitcast(F32R)
    with tc.tile_pool(name="ln_s", bufs=1) as spool, \
         tc.tile_pool(name="ln_tmp", bufs=3) as tmp_pool, \
         tc.tile_pool(name="ln_ps", bufs=2, space="PSUM") as pp_stat, \
         tc.tile_pool(name="ln_bc", bufs=2, space="PSUM") as pp_bc:
        ps_sum = pp_stat.tile([1, n], F32, tag="stat")
        for j in range(nd):
            nc.tensor.matmul(ps_sum[:], ones_col, _r(x_t[j][:]),
                             start=(j == 0), stop=(j == nd - 1))
        ps_sq = pp_stat.tile([1, n], F32, tag="stat")
        for j in range(nd):
            sq = tmp_pool.tile([128, n], sq_dt, tag="ln_sq")
            with nc.allow_low_precision(reason="bf16 sq for LN stats"):
                nc.scalar.activation(sq[:], x_t[j][:], AF.Square)
            nc.tensor.matmul(ps_sq[:], ones_col, sq[:],
                             start=(j == 0), stop=(j == nd - 1))
        mean = spool.tile([1, n], F32R, tag="ln_mean")
        nc.scalar.activation(mean[:], ps_sum[:], AF.Copy,
                             scale=1.0 / (nd * 128))
        msq = spool.tile([1, n], F32, tag="ln_msq")
        nc.scalar.activation(msq[:], ps_sq[:], AF.Copy,
                             scale=1.0 / (nd * 128))
        m2 = spool.tile([1, n], F32, tag="ln_m2")
        nc.vector.tensor_mul(m2[:], mean[:], mean[:])
        var = spool.tile([1, n], F32, tag="ln_var")
        nc.vector.tensor_sub(var[:], msq[:], m2[:])
        std = spool.tile([1, n], F32, tag="ln_std")
        nc.scalar.activation(std[:], var[:], AF.Sqrt, bias=eps_t[:])
        rstd = spool.tile([1, n], F32R, tag="ln_rstd")
        with nc.allow_low_precision(reason="rstd feeds fp32r bcast matmul"):
            nc.vector.reciprocal(rstd[:], std[:])
        ps_mb = pp_bc.tile([128, n], F32, tag="bcast")
        nc.tensor.matmul(ps_mb[:], ones[0:1, :].bitcast(F32R), mean[:],
                         start=True, stop=True)
        ps_rb = pp_bc.tile([128, n], F32, tag="bcast")
        nc.tensor.matmul(ps_rb[:], ones[0:1, :].bitcast(F32R), rstd[:],
                         start=True, stop=True)
        h_t = []
        for j in range(nd):
            xc = tmp_pool.tile([128, n], F32, tag="ln_xc")
            nc.vector.tensor_sub(xc[:], x_t[j][:], ps_mb[:])
            xn = tmp_pool.tile([128, n], F32, tag="ln_xn")
            nc.vector.tensor_mul(xn[:], xc[:], ps_rb[:])
            h = out_pool.tile([128, n], BF16, tag=out_tag)
            with nc.allow_low_precision(reason="bf16 gemm operands"):
                nc.scalar.activation(h[:], xn[:], AF.Identity,
                                     bias=shift_fn(j), scale=scale_fn(j))
            h_t.append(h)
        return h_t


def _proj_mtiles(nc, wimg_pool, pp, img_ap, h_t, out_cb, tag, nm=NDP):
    """m-tile projection: out[m] = sum_d w[d,m]^T h[d]; out_cb(m, ps)."""
    for m in range(nm):
        wt = wimg_pool.tile([128, DIM], BF16, tag=tag)
        nc.sync.dma_start(wt[:], img_ap[m])
        ps = pp.tile([128, T], F32, tag="mm")
        for d in range(ND):
            nc.tensor.matmul(ps[:], wt[:, d * 128:(d + 1) * 128],
                             h_t[d][:], start=(d == 0), stop=(d == ND - 1))
        out_cb(m, ps)


def _out_proj(nc, pp, stage, xpool, wimg_pool, o_pk, wo_img, bias_col,
              x_prev_fn, x_tag):
    """Attn out-projection from packed o tiles + bias + residual."""
    x_new = []
    for m in range(ND):
        wt = wimg_pool.tile([128, NDP * 128], BF16, tag="wimg")
        nc.sync.dma_start(wt[:], wo_img[m])
        ps = pp.tile([128, T], F32, tag="mm")
        for b in range(NDP):
            nc.tensor.matmul(ps[:], wt[:, b * 128:(b + 1) * 128],
                             o_pk[b][:], start=(b == 0),
                             stop=(b == NDP - 1))
        t1 = stage.tile([128, T], F32, tag="t1")
        nc.scalar.activation(t1[:], ps[:], AF.Identity, bias=bias_col(m))
        xn = xpool.tile([128, T], F32R, tag=x_tag)
        with nc.allow_low_precision(reason="residual stream fp32r"):
            nc.vector.tensor_add(xn[:], t1[:], x_prev_fn(m))
        x_new.append(xn)
    return x_new


def _attn_core(nc, tc, pools, kslice_a, kslice_b, vslice_a, vslice_b,
               q_a, q_b, nkt, kpart, e_tag):
    """Shared attention inner loop: per head scores->exp->PV->normalize.
    Returns packed o tiles (8 a-blocks + 2 b-blocks), bf16."""
    (opool, epool, apool, pp, pp_pva, pp_pvb, pp_bc, ones) = pools
    o_pk = [opool.tile([128, T], BF16, tag=f"opk{i}", name=f"opk{i}")
            for i in range(8)]
    o_pkb = [opool.tile([128, T], BF16, tag=f"opkb{i}", name=f"opkb{i}")
             for i in range(3)]
    nc.vector.memset(o_pkb[0][96:128, :], 0.0)
    nc.vector.memset(o_pkb[1][96:128, :], 0.0)
    nc.vector.memset(o_pkb[2][64:128, :], 0.0)
    def normalize(h, ps_a, ps_b):
        rt = apool.tile([33, T], F32R, tag="recip")
        with nc.allow_low_precision(reason="softmax recip"):
            nc.vector.reciprocal(rt[32:33, :], ps_b[32:33, :])
        ps_rb = pp_bc.tile([128, T], F32, tag="bcast")
        nc.tensor.matmul(ps_rb[:], ones[32:33, :].bitcast(F32R),
                         rt[32:33, :], start=True, stop=True)
        rb = apool.tile([128, T], F32, tag="rb")
        nc.scalar.copy(rb[:], ps_rb[:])
        blk, r0 = _bslot(h)
        with nc.allow_low_precision(reason="attn out bf16"):
            nc.vector.tensor_mul(o_pk[h][:], ps_a[:], rb[:])
            nc.vector.tensor_mul(o_pkb[blk - 8][r0:r0 + 32, :],
                                 ps_b[0:32, :], rb[r0:r0 + 32, :])

    LAG = 4
    for h in range(HEADS):
        e_t = []
        ps_a = pp_pva.tile([128, T], F32, tag="pva")
        ps_b = pp_pvb.tile([128, T], F32, tag="pvb")

        def emit_pv(kt, ps_a=ps_a, ps_b=ps_b, e_t=e_t, h=h):
            nc.tensor.matmul(ps_a[:], vslice_a(h, kt), e_t[kt][:],
                             start=(kt == 0), stop=(kt == nkt - 1))
            nc.tensor.matmul(ps_b[:], vslice_b(h, kt), e_t[kt][:],
                             start=(kt == 0), stop=(kt == nkt - 1))

        for kt in range(nkt):
            ps = pp.tile([kpart, T], F32, tag="mm")
            nc.tensor.matmul(ps[:], kslice_a(h, kt), q_a(h),
                             start=True, stop=False)
            nc.tensor.matmul(ps[:], kslice_b(h, kt), q_b(h),
                             start=False, stop=True)
            ex = epool.tile([kpart, T], BF16, tag=e_tag)
            with nc.allow_low_precision(reason="bf16 probs"):
                nc.scalar.activation(ex[:], ps[:], AF.Exp, scale=ATT_SCALE)
            e_t.append(ex)
            if kt >= LAG:
                emit_pv(kt - LAG)
        for kt in range(max(0, nkt - LAG), nkt):
            emit_pv(kt)
        normalize(h, ps_a, ps_b)
    return o_pk + o_pkb


# --------------------------------------------------------------------------
# the single-launch program
# --------------------------------------------------------------------------

def _build():
    nc = bacc.Bacc("TRN2", target_bir_lowering=False, debug=False,
                   num_devices=NCORES)
    P = nc.declare_dram_parameter
    t = {}
    t["xT"] = P("xT", [DIM, T], BF16, isOutput=False)
    t["ctxT"] = P("ctxT", [CROSS, SCP], BF16, isOutput=False)
    t["emb_img"] = P("emb_img", [128, 2 * ND], F32, isOutput=False)
    t["ada_w_img"] = P("ada_w_img", [ND, 128, ND * 512], BF16,
                       isOutput=False)
    t["ada_b_img"] = P("ada_b_img", [4 * DIM], F32, isOutput=False)
    t["wq_img"] = P("wq_img", [NDP, 128, DIM], BF16, isOutput=False)
    t["wk_img"] = P("wk_img", [NDP, 128, DIM], BF16, isOutput=False)
    t["wv_img"] = P("wv_img", [ND, 128, VW], BF16, isOutput=False)
    t["wo1_img"] = P("wo1_img", [ND, 128, NDP * 128], BF16, isOutput=False)
    t["wq2_img"] = P("wq2_img", [NDP, 128, DIM], BF16, isOutput=False)
    t["wk2_img"] = P("wk2_img", [NDP, 128, CROSS], BF16, isOutput=False)
    t["wv2_img"] = P("wv2_img", [NDC, 128, VW], BF16, isOutput=False)
    t["wo2_img"] = P("wo2_img", [ND, 128, NDP * 128], BF16, isOutput=False)
    t["biases_img"] = P("biases_img", [128, 4 * ND], F32, isOutput=False)
    t["w1_img"] = P("w1_img", [NM1, 128, DIM], BF16, isOutput=False)
    t["b1_img"] = P("b1_img", [128, NM1], F32, isOutput=False)
    t["w2_img"] = P("w2_img", [ND, 128, INNER], BF16, isOutput=False)
    t["b2_img"] = P("b2_img", [128, ND], F32, isOutput=False)
    t["yT"] = P("yT", [DIM, T], F32, isOutput=True)

    with tile.TileContext(nc) as tc:
        _kernel_body(nc, tc, t)
    nc.compile()
    return nc


def _kernel_body(nc, tc, t):
    import contextlib
    with contextlib.ExitStack() as es:
        e = es.enter_context
        cpool = e(tc.tile_pool(name="const", bufs=1))
        spool = e(tc.tile_pool(name="spool", bufs=1))
        stage = e(tc.tile_pool(name="stage", bufs=2))
        xpool3 = e(tc.tile_pool(name="xp3", bufs=ND))
        dram_pool = e(tc.tile_pool(name="dram", bufs=1, space="DRAM"))

        ones, ones_bf, eps_t = _consts(nc, cpool)
        biases = cpool.tile([128, 4 * ND], F32, tag="biases")
        nc.sync.dma_start(biases[:], t["biases_img"][:])
        b1_t = cpool.tile([128, NM1], F32, tag="b1")
        nc.sync.dma_start(b1_t[:], t["b1_img"][:])
        b2_t = cpool.tile([128, ND], F32, tag="b2")
        nc.sync.dma_start(b2_t[:], t["b2_img"][:])

        x0 = cpool.tile([128, ND * T], BF16, tag="x0")
        nc.sync.dma_start(x0[:].rearrange("p (b t) -> p b t", t=T),
                          t["xT"][:].rearrange("(b p) t -> p b t", p=128))
        x_t = [x0[:, j * T:(j + 1) * T] for j in range(ND)]

        # ---------------- cross-attn context K/V (independent; early) ------
        ctx_sb = cpool.tile([128, NDC * SCP], BF16, tag="ctx")
        nc.sync.dma_start(ctx_sb[:].rearrange("p (d s) -> p d s", s=SCP),
                          t["ctxT"][:].rearrange("(d p) s -> p d s", p=128))
        ctx_t = [ctx_sb[:, d * SCP:(d + 1) * SCP] for d in range(NDC)]
        # ---------------- adaLN embeddings ----------------
        s2d1, onep1, s2d2, onep2 = _ada_local(
            nc, tc, t["emb_img"], t["ada_w_img"], t["ada_b_img"],
            spool, dram_pool)

        KSP = 6                       # K AG split: blocks 0-5 / 6-10
        kstg = [dram_pool.tile([KSP * 128, T], BF16, name="kstg0"),
                dram_pool.tile([(NDP - KSP) * 128, T], BF16, name="kstg1")]
        vstg = [dram_pool.tile([2 * 128, VW], BF16, name="vstg0"),
                dram_pool.tile([2 * 128, VW], BF16, name="vstg1")]
        kgth = [dram_pool.tile([GROUP, KSP * 128, T], BF16, name="kgth0"),
                dram_pool.tile([GROUP, (NDP - KSP) * 128, T], BF16,
                               name="kgth1")]
        vgth = [dram_pool.tile([GROUP, 2 * 128, VW], BF16, name="vgth0"),
                dram_pool.tile([GROUP, 2 * 128, VW], BF16, name="vgth1")]
        groups = [[0, 1, 2, 3], [4, 5, 6, 7]]

        # ---------------- LN1 + q/k/v projections ----------------
        xp2_es = contextlib.ExitStack()
        xpool2 = xp2_es.enter_context(tc.tile_pool(name="xp2", bufs=ND))
        kvq_es = contextlib.ExitStack()
        q_sb = kvq_es.enter_context(
            tc.tile_pool(name="qp", bufs=1)).tile(
            [128, NDP * T], BF16, tag="q")
        with tc.tile_pool(name="hp", bufs=ND) as hpool, \
             tc.tile_pool(name="wimg", bufs=3) as wimg, \
             tc.tile_pool(name="kv1", bufs=1) as kv1pool, \
             tc.tile_pool(name="kv", bufs=2) as kvpool, \
             tc.tile_pool(name="ppa", bufs=3, space="PSUM") as pp:
            h_t = _layernorm(nc, tc, x_t, T,
                             lambda j: onep1[:, j:j + 1],
                             lambda j: s2d1[:, ND + j:ND + j + 1],
                             hpool, "h", ones, eps_t,
                             ones_stat=ones_bf[:, 0:1], sq_dt=BF16)

            k_all = kv1pool.tile([128, NDP * T], BF16, tag="kall")

            def k_out(m, ps):
                with nc.allow_low_precision(reason="bf16 k"):
                    nc.vector.tensor_copy(k_all[:, m * T:(m + 1) * T],
                                          ps[:])

            for m in range(NDP):
                wt = wimg.tile([128, DIM], BF16, tag="wimg")
                nc.sync.dma_start(wt[:], t["wk_img"][m])
                ps = pp.tile([128, T], F32, tag="mm")
                for d in range(ND):
                    nc.tensor.matmul(ps[:], wt[:, d * 128:(d + 1) * 128],
                                     h_t[d][:], start=(d == 0),
                                     stop=(d == ND - 1))
                k_out(m, ps)
                if m == KSP - 1 or m == NDP - 1:
                    half = 0 if m == KSP - 1 else 1
                    c0 = 0 if half == 0 else KSP * T
                    nb = KSP if half == 0 else NDP - KSP
                    nc.sync.dma_start(
                        kstg[half][:].rearrange("(b p) t -> p b t", p=128),
                        k_all[:, c0:c0 + nb * T]
                        .rearrange("p (b t) -> p b t", t=T))
                    nc.gpsimd.collective_compute(
                        "AllGather", mybir.AluOpType.bypass,
                        replica_groups=groups, ins=[kstg[half][:]],
                        outs=[kgth[half][:]])

            wv_sb = kv1pool.tile([128, ND * VW], BF16, tag="wv")
            nc.sync.dma_start(
                wv_sb[:].rearrange("p (d c) -> p d c", c=VW),
                t["wv_img"][:].rearrange("d p c -> p d c"))
            def q_out(m, ps):
                with nc.allow_low_precision(reason="bf16 q"):
                    nc.vector.tensor_copy(q_sb[:, m * T:(m + 1) * T],
                                          ps[:])

            _proj_mtiles(nc, wimg, pp, t["wq_img"], h_t, q_out, "wimg")
            for tt in range(NTT):
                vtile = kvpool.tile([128, VW], BF16, tag="vtile")
                for off, nn in ((0, 512), (512, 512), (1024, VW - 1024)):
                    ps = pp.tile([128, 512], F32, tag="mm")
                    for d in range(ND):
                        nc.tensor.matmul(
                            ps[:, 0:nn], h_t[d][:, tt * 128:(tt + 1) * 128],
                            wv_sb[:, d * VW + off:d * VW + off + nn],
                            start=(d == 0), stop=(d == ND - 1))
                    with nc.allow_low_precision(reason="bf16 v"):
                        nc.scalar.copy(vtile[:, off:off + nn], ps[:, 0:nn])
                for h in range(HEADS):
                    nc.vector.memset(vtile[:, h * DHP + DH:(h + 1) * DHP],
                                     1.0)
                half, r = tt // 2, (tt % 2) * 128
                nc.sync.dma_start(vstg[half][r:r + 128, :], vtile[:])
                if tt % 2 == 1:
                    nc.gpsimd.collective_compute(
                        "AllGather", mybir.AluOpType.bypass,
                        replica_groups=groups, ins=[vstg[half][:]],
                        outs=[vgth[half][:]])


        # ---------------- cross-attn context K/V (fills the AG gap) --
        k2_sb = cpool.tile([128, NDP * SCP], BF16, tag="k2")
        v2pad = cpool.tile([SCP, VW + 128], BF16, tag="v2pad")
        nc.vector.memset(v2pad[:], 0.0)
        with tc.tile_pool(name="cw", bufs=3) as cwpool, \
             tc.tile_pool(name="cwv1", bufs=1) as cwvpool, \
             tc.tile_pool(name="cps", bufs=2, space="PSUM") as cpp:
            for m in range(NDP):
                wt = cwpool.tile([128, CROSS], BF16, tag="cw")
                nc.sync.dma_start(wt[:], t["wk2_img"][m])
                ps = cpp.tile([128, SCP], F32, tag="cmm")
                for d in range(NDC):
                    nc.tensor.matmul(ps[:], wt[:, d * 128:(d + 1) * 128],
                                     ctx_t[d], start=(d == 0),
                                     stop=(d == NDC - 1))
                with nc.allow_low_precision(reason="bf16 k2"):
                    nc.scalar.copy(k2_sb[:, m * SCP:(m + 1) * SCP], ps[:])
            wv2_sb = cwvpool.tile([128, NDC * VW], BF16, tag="cwv")
            nc.sync.dma_start(
                wv2_sb[:].rearrange("p (d c) -> p d c", c=VW),
                t["wv2_img"][:].rearrange("d p c -> p d c"))
            for off, nn in ((0, 512), (512, 512), (1024, VW - 1024)):
                ps = cpp.tile([SCP, 512], F32, tag="cmm2")
                for d in range(NDC):
                    nc.tensor.matmul(ps[:, 0:nn], ctx_t[d],
                                     wv2_sb[:, d * VW + off:d * VW + off + nn],
                                     start=(d == 0), stop=(d == NDC - 1))
                with nc.allow_low_precision(reason="bf16 v2"):
                    nc.scalar.copy(v2pad[:, off:off + nn], ps[:, 0:nn])
            # ones columns for the denominator (rows 77:80 stay zero: ctx
            # padding is zero so the psum wrote zeros there, and the host
            # zeroed the wv2 ones-columns)
            for h in range(HEADS):
                nc.vector.memset(v2pad[0:SCTX, h * DHP + DH:(h + 1) * DHP],
                                 1.0)

        # ---------------- self-attention ----------------
        with tc.tile_pool(name="att_o", bufs=1) as opool, \
             tc.tile_pool(name="att_e", bufs=10) as epool, \
             tc.tile_pool(name="att_s", bufs=2) as apool, \
             tc.tile_pool(name="wimg", bufs=3) as wimg, \
             tc.tile_pool(name="ppa", bufs=3, space="PSUM") as pp, \
             tc.tile_pool(name="ppv", bufs=2, space="PSUM") as pp_pva, \
             tc.tile_pool(name="ppw", bufs=2, space="PSUM") as pp_pvb, \
             tc.tile_pool(name="ppb", bufs=1, space="PSUM") as pp_bc:
            gkv_es = contextlib.ExitStack()
            gkvpool = gkv_es.enter_context(tc.tile_pool(name="gkv", bufs=1))
            kt_sb = [gkvpool.tile([128, NDP * T], BF16, tag=f"ktg{g}",
                                  name=f"ktg{g}") for g in range(GROUP)]
            vt_sb = [gkvpool.tile([128, NTT * VW + 128], BF16,
                                  tag=f"vtg{g}", name=f"vtg{g}")
                     for g in range(GROUP)]
            for g in range(GROUP):
                nc.vector.memset(vt_sb[g][:, NTT * VW:], 0.0)
            for g in range(GROUP):
                nc.sync.dma_start(
                    kt_sb[g][:, 0:KSP * T]
                    .rearrange("p (b t) -> p b t", t=T),
                    kgth[0][g].rearrange("(b p) t -> p b t", p=128))
                nc.sync.dma_start(
                    kt_sb[g][:, KSP * T:]
                    .rearrange("p (b t) -> p b t", t=T),
                    kgth[1][g].rearrange("(b p) t -> p b t", p=128))
            for g in range(GROUP):
                nc.sync.dma_start(
                    vt_sb[g][:, 0:2 * VW]
                    .rearrange("p (j c) -> p j c", c=VW),
                    vgth[0][g].rearrange("(j p) c -> p j c", p=128))
                nc.sync.dma_start(
                    vt_sb[g][:, 2 * VW:4 * VW]
                    .rearrange("p (j c) -> p j c", c=VW),
                    vgth[1][g].rearrange("(j p) c -> p j c", p=128))

            def kslice_a(h, kt):
                g, j = kt % GROUP, kt // GROUP
                return kt_sb[g][:, h * T + j * 128:h * T + (j + 1) * 128]

            qb_pad = gkvpool.tile([128, HEADS * T], BF16, tag="qbp")
            nc.vector.memset(qb_pad[:], 0.0)
            for h in range(HEADS):
                blk, r0 = _bslot(h)
                with nc.allow_low_precision(reason="bf16 qb"):
                    nc.vector.tensor_copy(
                        qb_pad[r0:r0 + 32, h * T:(h + 1) * T],
                        q_sb[r0:r0 + 32, blk * T:(blk + 1) * T])

            def kslice_b(h, kt):
                g, j = kt % GROUP, kt // GROUP
                blk, r0 = _bslot(h)
                return kt_sb[g][:, blk * T + j * 128:blk * T + (j + 1) * 128]

            def vslice_a(h, kt):
                g, j = kt % GROUP, kt // GROUP
                return vt_sb[g][:, j * VW + h * DHP:j * VW + h * DHP + 128]

            def vslice_b(h, kt):
                g, j = kt % GROUP, kt // GROUP
                return vt_sb[g][:, j * VW + h * DHP + 128:
                                j * VW + h * DHP + 256]

            def q_a(h):
                return q_sb[:, h * T:(h + 1) * T]

            def q_b(h):
                return qb_pad[:, h * T:(h + 1) * T]

            pools = (opool, epool, apool, pp, pp_pva, pp_pvb, pp_bc, ones)
            o_pk = _attn_core(nc, tc, pools, kslice_a, kslice_b,
                              vslice_a, vslice_b, q_a, q_b, NKT, 128, "e1")
            gkv_es.close()
            x2_t = _out_proj(nc, pp, stage, xpool2, wimg, o_pk,
                             t["wo1_img"],
                             lambda m: biases[:, m:m + 1],
                             lambda m: x_t[m], "x2")
        kvq_es.close()

        # ---------------- cross-attention ----------------
        q2_es = contextlib.ExitStack()
        q2_sb = q2_es.enter_context(
            tc.tile_pool(name="q2p", bufs=1)).tile(
            [128, NDP * T], BF16, tag="q2")
        with tc.tile_pool(name="hp", bufs=ND) as hpool, \
             tc.tile_pool(name="wimg", bufs=3) as wimg, \
             tc.tile_pool(name="cr_o", bufs=1) as opool, \
             tc.tile_pool(name="cr_e", bufs=4) as epool, \
             tc.tile_pool(name="cr_s", bufs=2) as apool:
            h2_t = _layernorm(nc, tc, x2_t, T,
                              lambda j: onep2[:, j:j + 1],
                              lambda j: s2d2[:, ND + j:ND + j + 1],
                              hpool, "h", ones, eps_t)

            def q2_out(m, ps):
                with nc.allow_low_precision(reason="bf16 q2"):
                    nc.vector.tensor_copy(q2_sb[:, m * T:(m + 1) * T],
                                          ps[:])

            with tc.tile_pool(name="ppq2", bufs=3, space="PSUM") as ppq2:
                _proj_mtiles(nc, wimg, ppq2, t["wq2_img"], h2_t, q2_out,
                             "wimg")

            def k2slice_a(h, kt):
                return k2_sb[:, h * SCP:(h + 1) * SCP]

            q2b_pad = opool.tile([128, HEADS * T], BF16, tag="q2bp")
            nc.vector.memset(q2b_pad[:], 0.0)
            for h in range(HEADS):
                blk, r0 = _bslot(h)
                with nc.allow_low_precision(reason="bf16 q2b"):
                    nc.vector.tensor_copy(
                        q2b_pad[r0:r0 + 32, h * T:(h + 1) * T],
                        q2_sb[r0:r0 + 32, blk * T:(blk + 1) * T])

            def k2slice_b(h, kt):
                blk, r0 = _bslot(h)
                return k2_sb[:, blk * SCP:(blk + 1) * SCP]

            def v2slice_a(h, kt):
                return v2pad[:, h * DHP:h * DHP + 128]

            def v2slice_b(h, kt):
                return v2pad[:, h * DHP + 128:h * DHP + 256]

            def q2_a(h):
                return q2_sb[:, h * T:(h + 1) * T]

            def q2_b(h):
                return q2b_pad[:, h * T:(h + 1) * T]

            with tc.tile_pool(name="ppa", bufs=2, space="PSUM") as pp, \
                 tc.tile_pool(name="ppv", bufs=2, space="PSUM") as pp_pva, \
                 tc.tile_pool(name="ppw", bufs=2, space="PSUM") as pp_pvb, \
                 tc.tile_pool(name="ppb", bufs=1, space="PSUM") as pp_bc:
                pools = (opool, epool, apool, pp, pp_pva, pp_pvb, pp_bc,
                         ones)
                o2_pk = _attn_core(nc, tc, pools, k2slice_a, k2slice_b,
                                   v2slice_a, v2slice_b, q2_a, q2_b, 1, SCP,
                                   "e2")
                x3_t = _out_proj(nc, pp, stage, xpool3, wimg, o2_pk,
                                 t["wo2_img"],
                                 lambda m: biases[:, ND + m:ND + m + 1],
                                 lambda m: x2_t[m][:], "x3")
        q2_es.close()
        xp2_es.close()

        # ---------------- GEGLU feed-forward ----------------
        with tc.tile_pool(name="hp", bufs=ND) as hpool, \
             tc.tile_pool(name="wimg", bufs=4) as wimg, \
             tc.tile_pool(name="ff_hg", bufs=NI) as hgpool, \
             tc.tile_pool(name="ff_u", bufs=3) as upool, \
             tc.tile_pool(name="ff_w2", bufs=2) as w2pool, \
             tc.tile_pool(name="ppa", bufs=4, space="PSUM") as pp:
            h3_t = _layernorm(nc, tc, x3_t, T,
                              lambda j: biases[:, 2 * ND + j:2 * ND + j + 1],
                              lambda j: biases[:, 3 * ND + j:3 * ND + j + 1],
                              hpool, "h", ones, eps_t)
            hg_t = []
            for i in range(NI):
                wt = wimg.tile([128, 2 * DIM], BF16, tag="w1pair")
                nc.sync.dma_start(
                    wt[:].rearrange("p (i c) -> p i c", c=DIM),
                    t["w1_img"][2 * i:2 * i + 2].rearrange("i p c -> p i c"))
                ps_u = pp.tile([128, T], F32, tag="mm")
                for d in range(ND):
                    nc.tensor.matmul(ps_u[:], wt[:, d * 128:(d + 1) * 128],
                                     h3_t[d][:], start=(d == 0),
                                     stop=(d == ND - 1))
                ps_g = pp.tile([128, T], F32, tag="mm")
                for d in range(ND):
                    nc.tensor.matmul(ps_g[:],
                                     wt[:, DIM + d * 128:DIM + (d + 1) * 128],
                                     h3_t[d][:], start=(d == 0),
                                     stop=(d == ND - 1))
                u = upool.tile([128, T], F32, tag="u")
                nc.scalar.activation(u[:], ps_u[:], AF.Identity,
                                     bias=b1_t[:, 2 * i:2 * i + 1])
                g = upool.tile([128, T], F32, tag="g")
                nc.scalar.activation(g[:], ps_g[:], AF.Gelu,
                                     bias=b1_t[:, 2 * i + 1:2 * i + 2])
                hg = hgpool.tile([128, T], BF16, tag="hg")
                with nc.allow_low_precision(reason="bf16 geglu"):
                    nc.vector.tensor_mul(hg[:], u[:], g[:])
                hg_t.append(hg)
            for m in range(ND):
                wt = w2pool.tile([128, INNER], BF16, tag="w2")
                nc.sync.dma_start(wt[:], t["w2_img"][m])
                ps = pp.tile([128, T], F32, tag="mm")
                for d in range(NI):
                    nc.tensor.matmul(ps[:], wt[:, d * 128:(d + 1) * 128],
                                     hg_t[d][:], start=(d == 0),
                                     stop=(d == NI - 1))
                t1 = stage.tile([128, T], F32, tag="t1")
                nc.scalar.activation(t1[:], ps[:], AF.Identity,
                                     bias=b2_t[:, m:m + 1])
                y = stage.tile([128, T], F32, tag="y")
                nc.vector.tensor_add(y[:], t1[:], x3_t[m][:])
                nc.sync.dma_start(t["yT"][m * 128:(m + 1) * 128, :], y[:])


# --------------------------------------------------------------------------
# host side: weight images
# --------------------------------------------------------------------------

def _head_perm_pad():
    """Padded column map [NDP*128]: 8 blocks of head dims 0-127, then 3
    blocks packing per-head 32-dim remainders at offsets 0/32/64; -1 = zero
    padding."""
    idx = np.full(NDP * 128, -1, np.int64)
    for h in range(HEADS):
        idx[h * 128:(h + 1) * 128] = np.arange(h * DH, h * DH + 128)
    for h in range(HEADS):
        blk, off = _bslot(h)
        idx[blk * 128 + off:blk * 128 + off + 32] = np.arange(
            h * DH + 128, h * DH + DH)
    return idx


def _pad_cols(w, idx):
    out = np.zeros((w.shape[0], len(idx)), w.dtype)
    sel = idx >= 0
    out[:, sel] = w[:, idx[sel]]
    return out


def _pad_rows(w, idx):
    out = np.zeros((len(idx), w.shape[1]), w.dtype)
    sel = idx >= 0
    out[sel, :] = w[idx[sel], :]
    return out


def _img_kxm(w, mcols=128):
    """[K, M] weight -> [M//mcols, 128, (K//128)*mcols] m-tile images."""
    K, M = w.shape
    nd, nm = K // 128, M // mcols
    return np.ascontiguousarray(
        w.reshape(nd, 128, nm, mcols).transpose(2, 1, 0, 3)
        .reshape(nm, 128, nd * mcols))


def _v_pad(w):
    """[K, DIM] -> [K, VW] with a zero column after each head's 160 dims."""
    K = w.shape[0]
    out = np.zeros((K, VW), w.dtype)
    for h in range(HEADS):
        out[:, h * DHP:h * DHP + DH] = w[:, h * DH:(h + 1) * DH]
    return out


def _col_img(v):
    """[N] -> [128, N//128] image: img[p, j] = v[j*128 + p]."""
    return np.ascontiguousarray(v.reshape(-1, 128).T)


_STATE = {}

_STATIC_NAMES = (
    "ada_w_img", "ada_b_img", "wq_img", "wk_img", "wv_img", "wo1_img",
    "wq2_img", "wk2_img", "wv2_img", "wo2_img", "biases_img",
    "w1_img", "b1_img", "w2_img", "b2_img",
)


def _prepare(inputs):
    key = tuple(np.asarray(inputs[k]).ctypes.data for k in
                ("a1_wq", "ff_w1", "ff_w2", "a2_wk", "a1_wo"))
    if _STATE.get("key") == key:
        return _STATE["prep"]
    f = np.float32
    idx = _head_perm_pad()
    g = {}

    def bimg(w):
        return _img_kxm(np.asarray(w, f)).astype(NPBF)

    g["wq_img"] = bimg(_pad_cols(np.asarray(inputs["a1_wq"], f), idx))
    g["wk_img"] = bimg(_pad_cols(np.asarray(inputs["a1_wk"], f), idx))
    g["wv_img"] = np.ascontiguousarray(
        _v_pad(np.asarray(inputs["a1_wv"], f)).reshape(ND, 128, VW)
    ).astype(NPBF)
    g["wo1_img"] = bimg(_pad_rows(np.asarray(inputs["a1_wo"], f), idx))
    g["wq2_img"] = bimg(_pad_cols(np.asarray(inputs["a2_wq"], f), idx))
    g["wk2_img"] = bimg(_pad_cols(np.asarray(inputs["a2_wk"], f), idx))
    g["wv2_img"] = np.ascontiguousarray(
        _v_pad(np.asarray(inputs["a2_wv"], f)).reshape(NDC, 128, VW)
    ).astype(NPBF)
    g["wo2_img"] = bimg(_pad_rows(np.asarray(inputs["a2_wo"], f), idx))

    a1w = np.asarray(inputs["ada1_w"], f)
    a1b = np.asarray(inputs["ada1_b"], f)
    a2w = np.asarray(inputs["ada2_w"], f)
    a2b = np.asarray(inputs["ada2_b"], f)
    g["ada_w_img"] = _img_kxm(np.hstack([a1w, a2w]),
                              mcols=512).astype(NPBF)
    g["ada_b_img"] = np.ascontiguousarray(np.concatenate([a1b, a2b]))

    g["biases_img"] = np.ascontiguousarray(np.concatenate([
        _col_img(np.asarray(inputs["a1_bo"], f)),
        _col_img(np.asarray(inputs["a2_bo"], f)),
        _col_img(np.asarray(inputs["norm3_g"], f)),
        _col_img(np.asarray(inputs["norm3_b"], f))], axis=1))

    w1 = np.asarray(inputs["ff_w1"], f)
    w1i = np.stack([w1[:, :INNER].reshape(DIM, NI, 128),
                    w1[:, INNER:].reshape(DIM, NI, 128)],
                   axis=2).reshape(DIM, 2 * INNER)
    g["w1_img"] = bimg(w1i)
    b1 = np.asarray(inputs["ff_b1"], f)
    b1i = np.stack([b1[:INNER].reshape(NI, 128),
                    b1[INNER:].reshape(NI, 128)], axis=1).reshape(-1)
    g["b1_img"] = _col_img(b1i)
    g["w2_img"] = bimg(np.asarray(inputs["ff_w2"], f))
    g["b2_img"] = _col_img(np.asarray(inputs["ff_b2"], f))
    _STATE["key"] = key
    _STATE["prep"] = g
    _STATE.pop("static_dev", None)
    return g


# --------------------------------------------------------------------------
# SPMD runner with device-resident static inputs
# --------------------------------------------------------------------------

class _SpmdRunner:
    """Caches the jitted callable and keeps device-resident global arrays
    for static inputs."""

    def __init__(self, nc, n_cores):
        import functools

        import jax
        import jax.numpy as jnp
        from jax.experimental.shard_map import shard_map
        from jax.sharding import Mesh, NamedSharding, PartitionSpec

        from concourse import bass2jax

        bass2jax.install_neuronx_cc_hook()
        self.jax = jax
        self.nc = nc
        self.n_cores = n_cores
        partition_name = (nc.partition_id_tensor.name
                          if nc.partition_id_tensor else None)
        in_names, out_names, out_avals, zero_shapes = [], [], [], []
        for alloc in nc.m.functions[0].allocations:
            if not isinstance(alloc, mybir.MemoryLocationSet):
                continue
            name = alloc.memorylocations[0].name
            if alloc.kind == "ExternalInput":
                if name != partition_name:
                    in_names.append(name)
            elif alloc.kind == "ExternalOutput":
                shape = tuple(alloc.tensor_shape)
                dtype = mybir.dt.np(alloc.dtype)
                out_names.append(name)
                out_avals.append(jax.core.ShapedArray(shape, dtype))
                zero_shapes.append((shape, dtype))
        self.n_params = len(in_names)
        self.in_names = list(in_names)
        self.out_names = list(out_names)
        self.out_avals = out_avals
        all_in_names = list(in_names) + list(out_names)
        if partition_name is not None:
            all_in_names.append(partition_name)
        donate = tuple(range(self.n_params,
                             self.n_params + len(out_names)))

        def _bdy(*args):
            operands = list(args)
            if partition_name is not None:
                operands.append(bass2jax.partition_id_tensor())
            outs = bass2jax._bass_exec_p.bind(
                *operands,
                out_avals=tuple(out_avals),
                in_names=tuple(all_in_names),
                out_names=tuple(out_names),
                lowering_input_output_aliases=(),
                sim_require_finite=True,
                sim_require_nnan=True,
                nc=nc,
            )
            return tuple(outs)

        devices = jax.devices()[:n_cores]
        self.mesh = Mesh(np.asarray(devices), ("core",))
        self.sharding = NamedSharding(self.mesh, PartitionSpec("core"))
        n_z = len(zero_shapes)
        self.sharded = jax.jit(
            shard_map(_bdy, mesh=self.mesh,
                      in_specs=(PartitionSpec("core"),) * (self.n_params
                                                           + n_z),
                      out_specs=(PartitionSpec("core"),) * len(out_names),
                      check_rep=False),
            donate_argnums=donate, keep_unused=True)
        self._zero_fns = []
        for shape, dtype in zero_shapes:
            gshape = (n_cores * shape[0],) + tuple(shape[1:])
            self._zero_fns.append(jax.jit(
                functools.partial(jnp.zeros, gshape, dtype),
                out_shardings=self.sharding))
        self._static_cache = {}

    def put_static(self, name, per_core_arrays):
        gl = np.concatenate(per_core_arrays, axis=0)
        self._static_cache[name] = self.jax.device_put(gl, self.sharding)

    def __call__(self, in_maps):
        args = []
        for name in self.in_names:
            if name in self._static_cache:
                args.append(self._static_cache[name])
            else:
                gl = np.concatenate(
                    [np.asarray(m[name]) for m in in_maps], axis=0)
                args.append(self.jax.device_put(gl, self.sharding))
        zeros = [zf() for zf in self._zero_fns]
        out_arrs = self.sharded(*args, *zeros)
        res = []
        for c in range(self.n_cores):
            res.append({
                name: np.asarray(out_arrs[i]).reshape(
                    self.n_cores, *self.out_avals[i].shape)[c]
                for i, name in enumerate(self.out_names)})
        return res


def kernel(**inputs):
    if "nc" not in _STATE:
        _STATE["nc"] = _build()
    g = _prepare(inputs)
    if "runner" not in _STATE:
        _STATE["runner"] = _SpmdRunner(_STATE["nc"], NCORES)
    runner = _STATE["runner"]
    if "static_dev" not in _STATE:
        for name in _STATIC_NAMES:
            arrs = g[name]
            if not isinstance(arrs, list):
                arrs = [arrs] * NCORES
            runner.put_static(name, arrs)
        _STATE["static_dev"] = True
    f = np.float32
    x = np.asarray(inputs["hidden_states"], f)
    ctx = np.asarray(inputs["context"], f)
    tstep = int(np.asarray(inputs["timestep"]))
    emb_img = np.ascontiguousarray(np.concatenate(
        [_col_img(np.asarray(inputs["ada1_emb"], f)[tstep]),
         _col_img(np.asarray(inputs["ada2_emb"], f)[tstep])], axis=1))

    xT = [np.ascontiguousarray(x[b].T) for b in range(B)]  # [DIM, S]
    xT_c = [np.ascontiguousarray(xT[c // GROUP][:, (c % GROUP) * T:
                                                (c % GROUP + 1) * T]
                                 ).astype(NPBF)
            for c in range(NCORES)]
    ctxT = []
    for b in range(B):
        cp = np.zeros((CROSS, SCP), f)
        cp[:, :SCTX] = ctx[b].T
        ctxT.append(cp.astype(NPBF))

    in_maps = [{
        "xT": xT_c[c], "ctxT": ctxT[c // GROUP], "emb_img": emb_img,
    } for c in range(NCORES)]
    res = runner(in_maps)

    y = np.empty((B, S, DIM), f)
    for c in range(NCORES):
        b, i = divmod(c, GROUP)
        y[b, i * T:(i + 1) * T, :] = res[c]["yT"].T
    return y
